# revision 1
# baseline (speedup 1.0000x reference)
"""EdgeConv block (kNN -> gather -> 1x1 conv -> GroupNorm -> ReLU -> max over k)
as a Bass/Tile kernel for 8 Trainium2 NeuronCores.

Problem shapes (hardcoded): B=4, C_IN=64, C_OUT=128, N=8192, K=16, G=8.

Sharding: core c handles batch b = c//2, query half h = c%2 (4096 queries),
with the batch's full key set replicated on both cores of the pair.
GroupNorm statistics are partial per core and combined with a pairwise
AllReduce on a [128, 2] tensor.

Math decomposition (avoids materializing [Nq, k, 2C] pair features):
  conv out[o,q,j] = W1 @ (nbr_j - Fi_q) + W2 @ Fi_q = A[o, idx[q,j]] + C[o,q]
  where A = W1 @ Fk  [O, Nk]  and  C = (W2 - W1) @ Fq  [O, Nq].
kNN scores s[q,p] = 2*Q.P - |P|^2 (monotone in -d2 per query) via fp32 PE
matmul with lhsT = [2qx; 2qy; 2qz; 1], rhs = [px; py; pz; -|P|^2].

Top-16 per query: 16 segments of Nk/16 keys; per-segment top-8 via DVE max8 +
max_index; merge the 16*8 candidates with two max8+match_replace rounds; turn
the selection mask into dense ranks with a prefix scan and compact the winning
global indices with a per-partition local_scatter.  (Exact unless >8 of the
true top-16 fall in one segment: P ~ 3e-6 per query.)

Neighbor reduction: gpsimd ap_gather of A columns (indices shared across all
128 channel partitions), then DVE blocked reduces for max_j / sum_j, fused
square-reduce for the GN second moment.
"""

from contextlib import ExitStack

import numpy as np

import concourse.bass as bass
import concourse.bacc as bacc
import concourse.mybir as mybir
from concourse.tile import TileContext
from concourse.bass_utils import run_bass_kernel_spmd

F32 = mybir.dt.float32
I16 = mybir.dt.int16
U16 = mybir.dt.uint16

B, C_IN, C_OUT, N_KEYS, KNN, G = 4, 64, 128, 8192, 16, 8
GN_EPS = 1e-5
N_CORES = 8


def build_edgeconv(nq, nk, nseg, n_pair_q, neg_gamma=False,
                   num_devices=N_CORES, use_cc=True):
    """Build the SPMD program. nq: queries per core; nk: keys per core;
    nseg: top-k segments (seg = nk//nseg <= 512); n_pair_q: total queries
    per batch across the core pair (GN denominator)."""
    seg = nk // nseg
    assert seg * nseg == nk and seg <= 512
    ncand = nseg * 8
    qtiles = nq // 128
    assert qtiles * 128 == nq
    chunk = min(256, nq)  # queries per gather chunk
    qstep = min(512, nq)
    nchunk = nq // chunk
    assert nchunk * chunk == nq
    gn_count = float(n_pair_q * KNN * (C_OUT // G))
    ngrp = num_devices // 2

    nc = bacc.Bacc("TRN2", target_bir_lowering=False, debug=False,
                   num_devices=num_devices)

    qt_ext = nc.dram_tensor("qt", [4, nq], F32, kind="ExternalInput")
    pt_ext = nc.dram_tensor("pt", [4, nk], F32, kind="ExternalInput")
    fk_ext = nc.dram_tensor("fk", [C_IN, nk], F32, kind="ExternalInput")
    fq_ext = nc.dram_tensor("fq", [C_IN, nq], F32, kind="ExternalInput")
    w1t_ext = nc.dram_tensor("w1t", [C_IN, C_OUT], F32, kind="ExternalInput")
    dt_ext = nc.dram_tensor("dt", [C_IN, C_OUT], F32, kind="ExternalInput")
    gam_ext = nc.dram_tensor("gam", [1, C_OUT], F32, kind="ExternalInput")
    bet_ext = nc.dram_tensor("bet", [1, C_OUT], F32, kind="ExternalInput")
    out_ext = nc.dram_tensor("out", [C_OUT, nq], F32, kind="ExternalOutput")

    idx_dram = nc.dram_tensor("idx_scratch", [nq, KNN], I16)
    row_dram = nc.dram_tensor("row_scratch", [2, C_OUT], F32)
    cc_in = nc.dram_tensor("cc_in", [C_OUT, 2], F32)
    cc_out = nc.dram_tensor("cc_out", [C_OUT, 2], F32)

    with TileContext(nc) as tc, ExitStack() as ctx:
        persist = ctx.enter_context(tc.tile_pool(name="persist", bufs=1))
        psum = ctx.enter_context(tc.tile_pool(name="psum", bufs=6,
                                              space="PSUM"))
        segp = ctx.enter_context(tc.tile_pool(name="segp", bufs=3))
        small = ctx.enter_context(tc.tile_pool(name="small", bufs=2))
        gchunk = ctx.enter_context(tc.tile_pool(name="gchunk", bufs=2))

        # ---- persistent SBUF ----
        # qt replicated at partition bases 0/32/64/96 so four q-tiles'
        # K=4 matmuls can run concurrently in distinct PE row groups
        rowtile = 4 if qtiles % 4 == 0 else 1
        qt_sb = persist.tile([128 if rowtile == 4 else 4, nq], F32,
                             tag="qt_sb")
        for r in range(rowtile):
            nc.sync.dma_start(out=qt_sb[32 * r:32 * r + 4, :],
                              in_=qt_ext[:, :])
        pt_sb = persist.tile([128 if rowtile == 4 else 4, nk], F32,
                             tag="pt_sb")
        for r in range(rowtile):
            nc.sync.dma_start(out=pt_sb[32 * r:32 * r + 4, :],
                              in_=pt_ext[:, :])
        w1t_sb = persist.tile([C_IN, C_OUT], F32, tag="w1t_sb")
        nc.sync.dma_start(out=w1t_sb, in_=w1t_ext[:, :])
        dtw_sb = persist.tile([C_IN, C_OUT], F32, tag="dtw_sb")
        nc.sync.dma_start(out=dtw_sb, in_=dt_ext[:, :])
        gam_sb = persist.tile([1, C_OUT], F32, tag="gam_sb")
        nc.sync.dma_start(out=gam_sb, in_=gam_ext[:, :])
        bet_sb = persist.tile([1, C_OUT], F32, tag="bet_sb")
        nc.sync.dma_start(out=bet_sb, in_=bet_ext[:, :])

        a_sb = persist.tile([C_OUT, nk], F32, tag="a_sb")
        c_sb = persist.tile([C_OUT, nq], F32, tag="c_sb")
        mpos_sb = persist.tile([C_OUT, nq], F32, tag="mpos_sb")
        mneg_sb = (persist.tile([C_OUT, nq], F32, tag="mneg_sb")
                   if neg_gamma else None)
        seg_off = persist.tile([128, ncand], I16, tag="seg_off")
        nc.gpsimd.iota(seg_off, pattern=[[seg, nseg], [0, 8]], base=0,
                       channel_multiplier=0)
        zeros_nc = persist.tile([128, ncand], F32, tag="zeros_nc")
        nc.vector.memset(zeros_nc, 0.0)

        # ---- A and C matmuls (fp32), feature inputs streamed in slices ----
        with tc.tile_pool(name="feat", bufs=3) as featp:
            for s0 in range(0, nk, 512):
                fk_t = featp.tile([C_IN, 512], F32, tag="fk_t")
                nc.sync.dma_start(out=fk_t, in_=fk_ext[:, s0:s0 + 512])
                ps = psum.tile([C_OUT, 512], F32, tag="ps")
                nc.tensor.matmul(ps, lhsT=w1t_sb, rhs=fk_t,
                                 start=True, stop=True)
                nc.scalar.copy(out=a_sb[:, s0:s0 + 512], in_=ps)
            for s0 in range(0, nq, qstep):
                fq_t = featp.tile([C_IN, qstep], F32, tag="fq_t")
                nc.sync.dma_start(out=fq_t, in_=fq_ext[:, s0:s0 + qstep])
                ps = psum.tile([C_OUT, qstep], F32, tag="ps")
                nc.tensor.matmul(ps, lhsT=dtw_sb, rhs=fq_t,
                                 start=True, stop=True)
                nc.scalar.copy(out=c_sb[:, s0:s0 + qstep], in_=ps)

        # stat accumulators (filled by interleaved gather chunks)
        r_sa = small.tile([128, 1], F32, tag="acc_sa")
        r_sqa = small.tile([128, 1], F32, tag="acc_sqa")
        r_csa = small.tile([128, 1], F32, tag="acc_csa")
        nc.vector.memset(r_sa, 0.0)
        nc.vector.memset(r_sqa, 0.0)
        nc.vector.memset(r_csa, 0.0)

        def emit_gather_chunk(ch):
            q0 = ch * chunk
            idxs_t = gchunk.tile([128, chunk], I16, tag="idxs_t")
            for g in range(8):
                nc.sync.dma_start(
                    out=idxs_t[g * 16:(g + 1) * 16, :],
                    in_=bass.AP(tensor=idx_dram, offset=q0 * KNN,
                                ap=[[1, KNN], [KNN, chunk]]),
                )
            ga = gchunk.tile([128, chunk * KNN], F32, tag="ga")
            nc.gpsimd.ap_gather(out_ap=ga, in_ap=a_sb, idxs_ap=idxs_t,
                                channels=128, num_elems=nk, d=1,
                                num_idxs=chunk * KNN)
            gav = ga.rearrange("p (q c) -> p q c", c=KNN)
            nc.vector.tensor_reduce(out=mpos_sb[:, q0:q0 + chunk], in_=gav,
                                    axis=mybir.AxisListType.X,
                                    op=mybir.AluOpType.max)
            if neg_gamma:
                nc.vector.tensor_reduce(out=mneg_sb[:, q0:q0 + chunk],
                                        in_=gav, axis=mybir.AxisListType.X,
                                        op=mybir.AluOpType.min)
            sa_c = gchunk.tile([128, chunk], F32, tag="sa_c")
            nc.vector.tensor_reduce(out=sa_c, in_=gav,
                                    axis=mybir.AxisListType.X,
                                    op=mybir.AluOpType.add)
            tmp1 = small.tile([128, 1], F32, tag="tmp1")
            nc.vector.tensor_reduce(out=tmp1, in_=sa_c,
                                    axis=mybir.AxisListType.X,
                                    op=mybir.AluOpType.add)
            nc.vector.tensor_add(r_sa, r_sa, tmp1)
            scr_c = gchunk.tile([128, chunk], F32, tag="scr_c")
            nc.vector.tensor_mul(scr_c, sa_c, c_sb[:, q0:q0 + chunk])
            nc.vector.tensor_reduce(out=tmp1, in_=scr_c,
                                    axis=mybir.AxisListType.X,
                                    op=mybir.AluOpType.add)
            nc.vector.tensor_add(r_csa, r_csa, tmp1)
            # in-place square on gpsimd (offloads the DVE bottleneck)
            nc.gpsimd.tensor_mul(ga, ga, ga)
            nc.vector.tensor_reduce(out=tmp1, in_=ga,
                                    axis=mybir.AxisListType.X,
                                    op=mybir.AluOpType.add)
            nc.vector.tensor_add(r_sqa, r_sqa, tmp1)

        queries_per_group = rowtile * 128
        # ---- per-q-tile kNN (row-tiled: `rowtile` q-tiles in flight),
        # with gather chunks interleaved as soon as their indices land ----
        for tq0 in range(0, qtiles, rowtile):
            cvs, cis = [], []
            for r in range(rowtile):
                cv_r = small.tile([128, ncand], F32, tag=f"cv{r}")
                ci_r = small.tile([128, ncand], U16, tag=f"ci{r}")
                cvs.append(cv_r)
                cis.append(ci_r)
            for s in range(nseg):
                for r in range(rowtile):
                    t = tq0 + r
                    lhs_q = qt_sb[32 * r:32 * r + 4,
                                  t * 128:(t + 1) * 128]
                    ps = psum.tile([128, seg], F32, tag="ps")
                    nc.tensor.matmul(ps, lhsT=lhs_q,
                                     rhs=pt_sb[32 * r:32 * r + 4,
                                               s * seg:(s + 1) * seg],
                                     start=True, stop=True,
                                     tile_position=(32 * r, 0))
                    ssb = segp.tile([128, seg], F32, tag="ssb")
                    nc.scalar.copy(out=ssb, in_=ps)
                    nc.vector.max(out=cvs[r][:, s * 8:(s + 1) * 8], in_=ssb)
                    nc.vector.max_index(out=cis[r][:, s * 8:(s + 1) * 8],
                                        in_max=cvs[r][:, s * 8:(s + 1) * 8],
                                        in_values=ssb)
            for r in range(rowtile):
                t = tq0 + r
                cv, ci = cvs[r], cis[r]
                v1 = small.tile([128, 8], F32, tag="v1")
                v2 = small.tile([128, 8], F32, tag="v2")
                cv2 = small.tile([128, ncand], F32, tag="cv2")
                cv3 = small.tile([128, ncand], F32, tag="cv3")
                nc.vector.max(out=v1, in_=cv)
                nc.vector.match_replace(out=cv2, in_to_replace=v1,
                                        in_values=cv, imm_value=-1e30)
                nc.vector.max(out=v2, in_=cv2)
                nc.vector.match_replace(out=cv3, in_to_replace=v2,
                                        in_values=cv2, imm_value=-1e30)
                maskf = small.tile([128, ncand], F32, tag="maskf")
                nc.vector.tensor_tensor(out=maskf, in0=cv, in1=cv3,
                                        op=mybir.AluOpType.not_equal)
                rk = small.tile([128, ncand], F32, tag="rk")
                nc.vector.tensor_tensor_scan(out=rk, data0=maskf,
                                             data1=zeros_nc, initial=0.0,
                                             op0=mybir.AluOpType.add,
                                             op1=mybir.AluOpType.add)
                tgt = small.tile([128, ncand], F32, tag="tgt")
                nc.vector.tensor_tensor(out=tgt, in0=rk, in1=maskf,
                                        op=mybir.AluOpType.mult)
                nc.vector.tensor_scalar_add(tgt, tgt, -1.0)
                tgt_i = small.tile([128, ncand], I16, tag="tgti")
                nc.vector.tensor_copy(tgt_i, tgt)
                gidx = small.tile([128, ncand], I16, tag="gidx")
                nc.vector.tensor_tensor(out=gidx, in0=ci.bitcast(I16),
                                        in1=seg_off, op=mybir.AluOpType.add)
                idx16 = small.tile([128, KNN], I16, tag="idx16")
                nc.gpsimd.local_scatter(out_ap=idx16, data_ap=gidx,
                                        idxs_ap=tgt_i, channels=128,
                                        num_elems=KNN, num_idxs=ncand)
                nc.sync.dma_start(out=idx_dram[t * 128:(t + 1) * 128, :],
                                  in_=idx16)
            # emit the PREVIOUS group's gather chunks: their idx writes
            # have had a full group of kNN work to complete, so the DRAM
            # round-trip latency is hidden
            if tq0 > 0:
                prev_q0 = (tq0 - rowtile) * 128
                for ch in range(prev_q0 // chunk,
                                (prev_q0 + queries_per_group) // chunk):
                    emit_gather_chunk(ch)

        # flush the final group's gather chunks
        last_q0 = (qtiles - rowtile) * 128
        for ch in range(last_q0 // chunk,
                        (last_q0 + queries_per_group) // chunk):
            emit_gather_chunk(ch)

        r_c = small.tile([128, 1], F32, tag="r_c")
        nc.vector.tensor_reduce(out=r_c, in_=c_sb,
                                axis=mybir.AxisListType.X,
                                op=mybir.AluOpType.add)
        r_c2 = small.tile([128, 1], F32, tag="r_c2")
        nc.vector.memset(r_c2, 0.0)
        tmpc = small.tile([128, 1], F32, tag="tmpc")
        for q0 in range(0, nq, qstep):
            scr5 = gchunk.tile([128, qstep], F32, tag="scr5")
            nc.vector.tensor_mul(scr5, c_sb[:, q0:q0 + qstep],
                                 c_sb[:, q0:q0 + qstep])
            nc.vector.tensor_reduce(out=tmpc, in_=scr5,
                                    axis=mybir.AxisListType.X,
                                    op=mybir.AluOpType.add)
            nc.vector.tensor_add(r_c2, r_c2, tmpc)

        s1p = small.tile([128, 1], F32, tag="s1p")
        nc.vector.tensor_scalar(out=s1p, in0=r_c, scalar1=float(KNN),
                                scalar2=None, op0=mybir.AluOpType.mult)
        nc.vector.tensor_add(s1p, s1p, r_sa)
        s2p = small.tile([128, 1], F32, tag="s2p")
        nc.vector.tensor_scalar(out=s2p, in0=r_c2, scalar1=float(KNN),
                                scalar2=None, op0=mybir.AluOpType.mult)
        t2 = small.tile([128, 1], F32, tag="t2")
        nc.vector.tensor_scalar(out=t2, in0=r_csa, scalar1=2.0,
                                scalar2=None, op0=mybir.AluOpType.mult)
        nc.vector.tensor_add(s2p, s2p, t2)
        nc.vector.tensor_add(s2p, s2p, r_sqa)

        # ---- pairwise allreduce of [128, 2] partials ----
        s12 = small.tile([128, 2], F32, tag="s12")
        nc.vector.tensor_copy(s12[:, 0:1], s1p)
        nc.vector.tensor_copy(s12[:, 1:2], s2p)
        nc.sync.dma_start(out=cc_in[:, :], in_=s12)
        if use_cc:
            nc.gpsimd.collective_compute(
                "AllReduce", mybir.AluOpType.add,
                replica_groups=[[2 * i, 2 * i + 1] for i in range(ngrp)],
                ins=[cc_in[:, :]], outs=[cc_out[:, :]])
        else:
            # diagnostic mode: no cross-core reduce (stats use only this
            # core's half; output is approximate)
            nc.sync.dma_start(out=cc_out[:, :], in_=s12)

        # ---- finish GroupNorm stats in [*, C_OUT] row layout ----
        st1 = small.tile([1, C_OUT], F32, tag="st1")
        nc.sync.dma_start(out=st1,
                          in_=bass.AP(tensor=cc_out, offset=0,
                                      ap=[[0, 1], [2, C_OUT]]))
        st2 = small.tile([1, C_OUT], F32, tag="st2")
        nc.sync.dma_start(out=st2,
                          in_=bass.AP(tensor=cc_out, offset=1,
                                      ap=[[0, 1], [2, C_OUT]]))
        sg1 = small.tile([1, G], F32, tag="sg1")
        nc.vector.tensor_reduce(out=sg1,
                                in_=st1.rearrange("p (g d) -> p g d", g=G),
                                axis=mybir.AxisListType.X,
                                op=mybir.AluOpType.add)
        sg2 = small.tile([1, G], F32, tag="sg2")
        nc.vector.tensor_reduce(out=sg2,
                                in_=st2.rearrange("p (g d) -> p g d", g=G),
                                axis=mybir.AxisListType.X,
                                op=mybir.AluOpType.add)
        mean_r = small.tile([1, G], F32, tag="mean_r")
        nc.vector.tensor_scalar(out=mean_r, in0=sg1,
                                scalar1=1.0 / gn_count, scalar2=None,
                                op0=mybir.AluOpType.mult)
        ex2_r = small.tile([1, G], F32, tag="ex2_r")
        nc.vector.tensor_scalar(out=ex2_r, in0=sg2,
                                scalar1=1.0 / gn_count, scalar2=None,
                                op0=mybir.AluOpType.mult)
        var_r = small.tile([1, G], F32, tag="var_r")
        nc.vector.tensor_tensor(out=var_r, in0=mean_r, in1=mean_r,
                                op=mybir.AluOpType.mult)
        nc.vector.tensor_tensor(out=var_r, in0=ex2_r, in1=var_r,
                                op=mybir.AluOpType.subtract)
        sd_r = small.tile([1, G], F32, tag="sd_r")
        nc.vector.tensor_scalar_add(var_r, var_r, GN_EPS)
        nc.scalar.activation(sd_r, var_r, mybir.ActivationFunctionType.Sqrt,
                             bias=0.0)
        rstd_r = small.tile([1, G], F32, tag="rstd_r")
        nc.vector.reciprocal(rstd_r, sd_r)
        mean_c = small.tile([1, C_OUT], F32, tag="mean_c")
        rstd_c = small.tile([1, C_OUT], F32, tag="rstd_c")
        gsz = C_OUT // G
        for g in range(G):
            nc.vector.tensor_copy(
                mean_c[:, g * gsz:(g + 1) * gsz],
                mean_r[:, g:g + 1].to_broadcast([1, gsz]))
            nc.vector.tensor_copy(
                rstd_c[:, g * gsz:(g + 1) * gsz],
                rstd_r[:, g:g + 1].to_broadcast([1, gsz]))
        srow = small.tile([1, C_OUT], F32, tag="srow")
        nc.vector.tensor_tensor(out=srow, in0=gam_sb, in1=rstd_c,
                                op=mybir.AluOpType.mult)
        trow = small.tile([1, C_OUT], F32, tag="trow")
        nc.vector.tensor_tensor(out=trow, in0=mean_c, in1=srow,
                                op=mybir.AluOpType.mult)
        nc.vector.tensor_tensor(out=trow, in0=bet_sb, in1=trow,
                                op=mybir.AluOpType.subtract)
        # transpose the two [1, C_OUT] rows to [C_OUT, 1] via DRAM bounce
        nc.sync.dma_start(out=row_dram[0:1, :], in_=srow)
        nc.sync.dma_start(out=row_dram[1:2, :], in_=trow)
        s_col = small.tile([C_OUT, 1], F32, tag="s_col")
        nc.sync.dma_start(out=s_col,
                          in_=bass.AP(tensor=row_dram, offset=0,
                                      ap=[[1, C_OUT], [0, 1]]))
        t_col = small.tile([C_OUT, 1], F32, tag="t_col")
        nc.sync.dma_start(out=t_col,
                          in_=bass.AP(tensor=row_dram, offset=C_OUT,
                                      ap=[[1, C_OUT], [0, 1]]))

        # ---- final normalization + relu + output ----
        for q0 in range(0, nq, qstep):
            mf = gchunk.tile([128, qstep], F32, tag="mf")
            nc.vector.tensor_add(mf, mpos_sb[:, q0:q0 + qstep],
                                 c_sb[:, q0:q0 + qstep])
            nc.vector.tensor_scalar(out=mf, in0=mf, scalar1=s_col,
                                    scalar2=t_col,
                                    op0=mybir.AluOpType.mult,
                                    op1=mybir.AluOpType.add)
            if neg_gamma:
                mn = gchunk.tile([128, qstep], F32, tag="mn")
                nc.vector.tensor_add(mn, mneg_sb[:, q0:q0 + qstep],
                                     c_sb[:, q0:q0 + qstep])
                nc.vector.tensor_scalar(out=mn, in0=mn, scalar1=s_col,
                                        scalar2=t_col,
                                        op0=mybir.AluOpType.mult,
                                        op1=mybir.AluOpType.add)
                nc.vector.tensor_tensor(out=mf, in0=mf, in1=mn,
                                        op=mybir.AluOpType.max)
            nc.vector.tensor_scalar_max(mf, mf, 0.0)
            nc.sync.dma_start(out=out_ext[:, q0:q0 + qstep], in_=mf)

    nc.finalize()
    return nc


def make_core_inputs(Fq, Fk, Pq, Pk, W, nq_half, core):
    b, h = core // 2, core % 2
    q0 = h * nq_half
    Qs = Pq[b][:, q0:q0 + nq_half]
    qt = np.concatenate([2.0 * Qs, np.ones((1, nq_half), np.float32)], 0)
    Pb = Pk[b]
    pt = np.concatenate([Pb, -(Pb * Pb).sum(0, keepdims=True)], 0)
    return {
        "qt": np.ascontiguousarray(qt, np.float32),
        "pt": np.ascontiguousarray(pt, np.float32),
        "fk": np.ascontiguousarray(Fk[b], np.float32),
        "fq": np.ascontiguousarray(Fq[b][:, q0:q0 + nq_half], np.float32),
        "w1t": np.ascontiguousarray(W[:, :C_IN].T, np.float32),
        "dt": np.ascontiguousarray((W[:, C_IN:] - W[:, :C_IN]).T, np.float32),
    }


_NC_CACHE = {}
TRACE = False       # set True to capture an NTFF profile on the next call
LAST_RESULT = None  # BassKernelResults of the most recent kernel() call


def kernel(Fq_bcn, Fk_bcn, Pq_b3n, Pk_b3n, W_conv, gn_gamma=None,
           gn_beta=None, k=16):
    k = int(k)
    assert k == KNN, f"kernel hardcodes k=16, got {k}"
    Fq = np.asarray(Fq_bcn, np.float32)
    Fk = np.asarray(Fk_bcn, np.float32)
    Pq = np.asarray(Pq_b3n, np.float32)
    Pk = np.asarray(Pk_b3n, np.float32)
    W = np.asarray(W_conv, np.float32)
    gam = (np.ones(C_OUT, np.float32) if gn_gamma is None
           else np.asarray(gn_gamma, np.float32).reshape(C_OUT))
    bet = (np.zeros(C_OUT, np.float32) if gn_beta is None
           else np.asarray(gn_beta, np.float32).reshape(C_OUT))
    assert Fq.shape == (B, C_IN, N_KEYS)

    nq = N_KEYS // 2
    neg = bool((gam < 0).any())
    key = ("full", neg)
    if key not in _NC_CACHE:
        _NC_CACHE[key] = build_edgeconv(nq=nq, nk=N_KEYS, nseg=16,
                                        n_pair_q=N_KEYS, neg_gamma=neg)
    nc = _NC_CACHE[key]

    in_maps = []
    for core in range(N_CORES):
        m = make_core_inputs(Fq, Fk, Pq, Pk, W, nq, core)
        m["gam"] = np.ascontiguousarray(gam.reshape(1, C_OUT))
        m["bet"] = np.ascontiguousarray(bet.reshape(1, C_OUT))
        in_maps.append(m)

    res = run_bass_kernel_spmd(nc, in_maps, core_ids=list(range(N_CORES)),
                               trace=TRACE)
    global LAST_RESULT
    LAST_RESULT = res
    out = np.empty((B, C_OUT, N_KEYS), np.float32)
    for core in range(N_CORES):
        b, h = core // 2, core % 2
        out[b, :, h * nq:(h + 1) * nq] = res.results[core]["out"]
    return out


if __name__ == "__main__":
    rng = np.random.default_rng(0)
    inputs = {
        "Fq_bcn": rng.standard_normal((B, C_IN, N_KEYS)).astype(np.float32),
        "Fk_bcn": rng.standard_normal((B, C_IN, N_KEYS)).astype(np.float32),
        "Pq_b3n": rng.standard_normal((B, 3, N_KEYS)).astype(np.float32),
        "Pk_b3n": rng.standard_normal((B, 3, N_KEYS)).astype(np.float32),
        "W_conv": (rng.standard_normal((C_OUT, 2 * C_IN)).astype(np.float32)
                   / np.sqrt(2 * C_IN)),
        "gn_gamma": np.ones(C_OUT, np.float32),
        "gn_beta": np.zeros(C_OUT, np.float32),
        "k": 16,
    }
    out = kernel(**inputs)
    print("kernel out", out.shape, out.dtype, float(np.abs(out).mean()))



# revision 3
# speedup vs baseline: 1.3207x; 1.3207x over previous
"""EdgeConv block (kNN -> gather -> 1x1 conv -> GroupNorm -> ReLU -> max over k)
as a Bass/Tile kernel for 8 Trainium2 NeuronCores.

Problem shapes (hardcoded): B=4, C_IN=64, C_OUT=128, N=8192, K=16, G=8.

Sharding: core c handles batch b = c//2, query half h = c%2 (4096 queries),
with the batch's full key set replicated on both cores of the pair.
GroupNorm statistics are partial per core and combined with a pairwise
AllReduce on a [128, 2] tensor.

Math decomposition (avoids materializing [Nq, k, 2C] pair features):
  conv out[o,q,j] = W1 @ (nbr_j - Fi_q) + W2 @ Fi_q = A[o, idx[q,j]] + C[o,q]
  where A = W1 @ Fk  [O, Nk]  and  C = (W2 - W1) @ Fq  [O, Nq].
kNN scores s[q,p] = 2*Q.P - |P|^2 (monotone in -d2 per query) via fp32 PE
matmul with lhsT = [2qx; 2qy; 2qz; 1], rhs = [px; py; pz; -|P|^2].

Top-16 per query: 16 segments of Nk/16 keys; per-segment top-8 via DVE max8 +
max_index; merge the 16*8 candidates with two max8+match_replace rounds; turn
the selection mask into dense ranks with a prefix scan and compact the winning
global indices with a per-partition local_scatter.  (Exact unless >8 of the
true top-16 fall in one segment: P ~ 3e-6 per query.)

Neighbor reduction: gpsimd ap_gather of A columns (indices shared across all
128 channel partitions), then DVE blocked reduces for max_j / sum_j, fused
square-reduce for the GN second moment.
"""

from contextlib import ExitStack

import numpy as np
import jax
from jax.sharding import Mesh, NamedSharding, PartitionSpec

import concourse.bass as bass
import concourse.bacc as bacc
import concourse.mybir as mybir
from concourse.tile import TileContext
from concourse import bass2jax

F32 = mybir.dt.float32
I16 = mybir.dt.int16
U16 = mybir.dt.uint16

B, C_IN, C_OUT, N_KEYS, KNN, G = 4, 64, 128, 8192, 16, 8
GN_EPS = 1e-5
N_CORES = 8


def build_edgeconv(nq, nk, nseg, n_pair_q, neg_gamma=False,
                   num_devices=N_CORES, use_cc=True):
    """Build the SPMD program. nq: queries per core; nk: keys per core;
    nseg: top-k segments (seg = nk//nseg <= 512); n_pair_q: total queries
    per batch across the core pair (GN denominator)."""
    seg = nk // nseg
    assert seg * nseg == nk and seg <= 512
    ncand = nseg * 8
    qtiles = nq // 128
    assert qtiles * 128 == nq
    chunk = min(256, nq)  # queries per gather chunk
    qstep = min(512, nq)
    nchunk = nq // chunk
    assert nchunk * chunk == nq
    gn_count = float(n_pair_q * KNN * (C_OUT // G))
    ngrp = num_devices // 2

    nc = bacc.Bacc("TRN2", target_bir_lowering=False, debug=False,
                   num_devices=num_devices)

    qt_ext = nc.dram_tensor("qt", [4, nq], F32, kind="ExternalInput")
    pt_ext = nc.dram_tensor("pt", [4, nk], F32, kind="ExternalInput")
    fk_ext = nc.dram_tensor("fk", [C_IN, nk], F32, kind="ExternalInput")
    fq_ext = nc.dram_tensor("fq", [C_IN, nq], F32, kind="ExternalInput")
    w1t_ext = nc.dram_tensor("w1t", [C_IN, C_OUT], F32, kind="ExternalInput")
    dt_ext = nc.dram_tensor("dt", [C_IN, C_OUT], F32, kind="ExternalInput")
    gam_ext = nc.dram_tensor("gam", [1, C_OUT], F32, kind="ExternalInput")
    bet_ext = nc.dram_tensor("bet", [1, C_OUT], F32, kind="ExternalInput")
    out_ext = nc.dram_tensor("out", [C_OUT, nq], F32, kind="ExternalOutput")

    idx_dram = nc.dram_tensor("idx_scratch", [nq, KNN], I16)
    row_dram = nc.dram_tensor("row_scratch", [2, C_OUT], F32)
    cc_in = nc.dram_tensor("cc_in", [C_OUT, 2], F32)
    cc_out = nc.dram_tensor("cc_out", [C_OUT, 2], F32)

    with TileContext(nc) as tc, ExitStack() as ctx:
        persist = ctx.enter_context(tc.tile_pool(name="persist", bufs=1))
        psum = ctx.enter_context(tc.tile_pool(name="psum", bufs=6,
                                              space="PSUM"))
        segp = ctx.enter_context(tc.tile_pool(name="segp", bufs=3))
        small = ctx.enter_context(tc.tile_pool(name="small", bufs=2))
        gchunk = ctx.enter_context(tc.tile_pool(name="gchunk", bufs=2))

        # ---- persistent SBUF ----
        # qt replicated at partition bases 0/32/64/96 so four q-tiles'
        # K=4 matmuls can run concurrently in distinct PE row groups
        rowtile = 4 if qtiles % 4 == 0 else 1
        qt_sb = persist.tile([128 if rowtile == 4 else 4, nq], F32,
                             tag="qt_sb")
        for r in range(rowtile):
            nc.sync.dma_start(out=qt_sb[32 * r:32 * r + 4, :],
                              in_=qt_ext[:, :])
        pt_sb = persist.tile([128 if rowtile == 4 else 4, nk], F32,
                             tag="pt_sb")
        for r in range(rowtile):
            nc.sync.dma_start(out=pt_sb[32 * r:32 * r + 4, :],
                              in_=pt_ext[:, :])
        w1t_sb = persist.tile([C_IN, C_OUT], F32, tag="w1t_sb")
        nc.sync.dma_start(out=w1t_sb, in_=w1t_ext[:, :])
        dtw_sb = persist.tile([C_IN, C_OUT], F32, tag="dtw_sb")
        nc.sync.dma_start(out=dtw_sb, in_=dt_ext[:, :])
        gam_sb = persist.tile([1, C_OUT], F32, tag="gam_sb")
        nc.sync.dma_start(out=gam_sb, in_=gam_ext[:, :])
        bet_sb = persist.tile([1, C_OUT], F32, tag="bet_sb")
        nc.sync.dma_start(out=bet_sb, in_=bet_ext[:, :])

        a_sb = persist.tile([C_OUT, nk], F32, tag="a_sb")
        c_sb = persist.tile([C_OUT, nq], F32, tag="c_sb")
        mpos_sb = persist.tile([C_OUT, nq], F32, tag="mpos_sb")
        mneg_sb = (persist.tile([C_OUT, nq], F32, tag="mneg_sb")
                   if neg_gamma else None)
        seg_off = persist.tile([128, ncand], I16, tag="seg_off")
        nc.gpsimd.iota(seg_off, pattern=[[seg, nseg], [0, 8]], base=0,
                       channel_multiplier=0)
        zeros_nc = persist.tile([128, ncand], F32, tag="zeros_nc")
        nc.vector.memset(zeros_nc, 0.0)

        # ---- A and C matmuls (fp32), feature inputs streamed in slices ----
        with tc.tile_pool(name="feat", bufs=3) as featp:
            for s0 in range(0, nk, 512):
                fk_t = featp.tile([C_IN, 512], F32, tag="fk_t")
                nc.sync.dma_start(out=fk_t, in_=fk_ext[:, s0:s0 + 512])
                ps = psum.tile([C_OUT, 512], F32, tag="ps")
                nc.tensor.matmul(ps, lhsT=w1t_sb, rhs=fk_t,
                                 start=True, stop=True)
                nc.scalar.copy(out=a_sb[:, s0:s0 + 512], in_=ps)
            for s0 in range(0, nq, qstep):
                fq_t = featp.tile([C_IN, qstep], F32, tag="fq_t")
                nc.sync.dma_start(out=fq_t, in_=fq_ext[:, s0:s0 + qstep])
                ps = psum.tile([C_OUT, qstep], F32, tag="ps")
                nc.tensor.matmul(ps, lhsT=dtw_sb, rhs=fq_t,
                                 start=True, stop=True)
                nc.scalar.copy(out=c_sb[:, s0:s0 + qstep], in_=ps)

        # stat accumulators (filled by interleaved gather chunks)
        r_sa = small.tile([128, 1], F32, tag="acc_sa")
        r_sqa = small.tile([128, 1], F32, tag="acc_sqa")
        r_csa = small.tile([128, 1], F32, tag="acc_csa")
        nc.vector.memset(r_sa, 0.0)
        nc.vector.memset(r_sqa, 0.0)
        nc.vector.memset(r_csa, 0.0)

        def emit_gather_chunk(ch):
            q0 = ch * chunk
            idxs_t = gchunk.tile([128, chunk], I16, tag="idxs_t")
            for g in range(8):
                nc.sync.dma_start(
                    out=idxs_t[g * 16:(g + 1) * 16, :],
                    in_=bass.AP(tensor=idx_dram, offset=q0 * KNN,
                                ap=[[1, KNN], [KNN, chunk]]),
                )
            ga = gchunk.tile([128, chunk * KNN], F32, tag="ga")
            nc.gpsimd.ap_gather(out_ap=ga, in_ap=a_sb, idxs_ap=idxs_t,
                                channels=128, num_elems=nk, d=1,
                                num_idxs=chunk * KNN)
            gav = ga.rearrange("p (q c) -> p q c", c=KNN)
            nc.vector.tensor_reduce(out=mpos_sb[:, q0:q0 + chunk], in_=gav,
                                    axis=mybir.AxisListType.X,
                                    op=mybir.AluOpType.max)
            if neg_gamma:
                nc.vector.tensor_reduce(out=mneg_sb[:, q0:q0 + chunk],
                                        in_=gav, axis=mybir.AxisListType.X,
                                        op=mybir.AluOpType.min)
            sa_c = gchunk.tile([128, chunk], F32, tag="sa_c")
            nc.vector.tensor_reduce(out=sa_c, in_=gav,
                                    axis=mybir.AxisListType.X,
                                    op=mybir.AluOpType.add)
            tmp1 = small.tile([128, 1], F32, tag="tmp1")
            nc.vector.tensor_reduce(out=tmp1, in_=sa_c,
                                    axis=mybir.AxisListType.X,
                                    op=mybir.AluOpType.add)
            nc.vector.tensor_add(r_sa, r_sa, tmp1)
            scr_c = gchunk.tile([128, chunk], F32, tag="scr_c")
            nc.vector.tensor_mul(scr_c, sa_c, c_sb[:, q0:q0 + chunk])
            nc.vector.tensor_reduce(out=tmp1, in_=scr_c,
                                    axis=mybir.AxisListType.X,
                                    op=mybir.AluOpType.add)
            nc.vector.tensor_add(r_csa, r_csa, tmp1)
            # in-place square on gpsimd (offloads the DVE bottleneck)
            nc.gpsimd.tensor_mul(ga, ga, ga)
            nc.vector.tensor_reduce(out=tmp1, in_=ga,
                                    axis=mybir.AxisListType.X,
                                    op=mybir.AluOpType.add)
            nc.vector.tensor_add(r_sqa, r_sqa, tmp1)

        queries_per_group = rowtile * 128
        # ---- per-q-tile kNN (row-tiled: `rowtile` q-tiles in flight),
        # with gather chunks interleaved as soon as their indices land ----
        for tq0 in range(0, qtiles, rowtile):
            cvs, cis = [], []
            for r in range(rowtile):
                cv_r = small.tile([128, ncand], F32, tag=f"cv{r}")
                ci_r = small.tile([128, ncand], U16, tag=f"ci{r}")
                cvs.append(cv_r)
                cis.append(ci_r)
            for s in range(nseg):
                for r in range(rowtile):
                    t = tq0 + r
                    lhs_q = qt_sb[32 * r:32 * r + 4,
                                  t * 128:(t + 1) * 128]
                    ps = psum.tile([128, seg], F32, tag="ps")
                    nc.tensor.matmul(ps, lhsT=lhs_q,
                                     rhs=pt_sb[32 * r:32 * r + 4,
                                               s * seg:(s + 1) * seg],
                                     start=True, stop=True,
                                     tile_position=(32 * r, 0))
                    ssb = segp.tile([128, seg], F32, tag="ssb")
                    nc.scalar.copy(out=ssb, in_=ps)
                    nc.vector.max(out=cvs[r][:, s * 8:(s + 1) * 8], in_=ssb)
                    nc.vector.max_index(out=cis[r][:, s * 8:(s + 1) * 8],
                                        in_max=cvs[r][:, s * 8:(s + 1) * 8],
                                        in_values=ssb)
            for r in range(rowtile):
                t = tq0 + r
                cv, ci = cvs[r], cis[r]
                v1 = small.tile([128, 8], F32, tag="v1")
                v2 = small.tile([128, 8], F32, tag="v2")
                cv2 = small.tile([128, ncand], F32, tag="cv2")
                cv3 = small.tile([128, ncand], F32, tag="cv3")
                nc.vector.max(out=v1, in_=cv)
                nc.vector.match_replace(out=cv2, in_to_replace=v1,
                                        in_values=cv, imm_value=-1e30)
                nc.vector.max(out=v2, in_=cv2)
                nc.vector.match_replace(out=cv3, in_to_replace=v2,
                                        in_values=cv2, imm_value=-1e30)
                maskf = small.tile([128, ncand], F32, tag="maskf")
                nc.vector.tensor_tensor(out=maskf, in0=cv, in1=cv3,
                                        op=mybir.AluOpType.not_equal)
                rk = small.tile([128, ncand], F32, tag="rk")
                nc.vector.tensor_tensor_scan(out=rk, data0=maskf,
                                             data1=zeros_nc, initial=0.0,
                                             op0=mybir.AluOpType.add,
                                             op1=mybir.AluOpType.add)
                tgt = small.tile([128, ncand], F32, tag="tgt")
                nc.vector.tensor_tensor(out=tgt, in0=rk, in1=maskf,
                                        op=mybir.AluOpType.mult)
                nc.vector.tensor_scalar_add(tgt, tgt, -1.0)
                tgt_i = small.tile([128, ncand], I16, tag="tgti")
                nc.vector.tensor_copy(tgt_i, tgt)
                gidx = small.tile([128, ncand], I16, tag="gidx")
                nc.vector.tensor_tensor(out=gidx, in0=ci.bitcast(I16),
                                        in1=seg_off, op=mybir.AluOpType.add)
                idx16 = small.tile([128, KNN], I16, tag="idx16")
                nc.gpsimd.local_scatter(out_ap=idx16, data_ap=gidx,
                                        idxs_ap=tgt_i, channels=128,
                                        num_elems=KNN, num_idxs=ncand)
                nc.sync.dma_start(out=idx_dram[t * 128:(t + 1) * 128, :],
                                  in_=idx16)
            # emit the PREVIOUS group's gather chunks: their idx writes
            # have had a full group of kNN work to complete, so the DRAM
            # round-trip latency is hidden
            if tq0 > 0:
                prev_q0 = (tq0 - rowtile) * 128
                for ch in range(prev_q0 // chunk,
                                (prev_q0 + queries_per_group) // chunk):
                    emit_gather_chunk(ch)

        # flush the final group's gather chunks
        last_q0 = (qtiles - rowtile) * 128
        for ch in range(last_q0 // chunk,
                        (last_q0 + queries_per_group) // chunk):
            emit_gather_chunk(ch)

        r_c = small.tile([128, 1], F32, tag="r_c")
        nc.vector.tensor_reduce(out=r_c, in_=c_sb,
                                axis=mybir.AxisListType.X,
                                op=mybir.AluOpType.add)
        r_c2 = small.tile([128, 1], F32, tag="r_c2")
        nc.vector.memset(r_c2, 0.0)
        tmpc = small.tile([128, 1], F32, tag="tmpc")
        for q0 in range(0, nq, qstep):
            scr5 = gchunk.tile([128, qstep], F32, tag="scr5")
            nc.vector.tensor_mul(scr5, c_sb[:, q0:q0 + qstep],
                                 c_sb[:, q0:q0 + qstep])
            nc.vector.tensor_reduce(out=tmpc, in_=scr5,
                                    axis=mybir.AxisListType.X,
                                    op=mybir.AluOpType.add)
            nc.vector.tensor_add(r_c2, r_c2, tmpc)

        s1p = small.tile([128, 1], F32, tag="s1p")
        nc.vector.tensor_scalar(out=s1p, in0=r_c, scalar1=float(KNN),
                                scalar2=None, op0=mybir.AluOpType.mult)
        nc.vector.tensor_add(s1p, s1p, r_sa)
        s2p = small.tile([128, 1], F32, tag="s2p")
        nc.vector.tensor_scalar(out=s2p, in0=r_c2, scalar1=float(KNN),
                                scalar2=None, op0=mybir.AluOpType.mult)
        t2 = small.tile([128, 1], F32, tag="t2")
        nc.vector.tensor_scalar(out=t2, in0=r_csa, scalar1=2.0,
                                scalar2=None, op0=mybir.AluOpType.mult)
        nc.vector.tensor_add(s2p, s2p, t2)
        nc.vector.tensor_add(s2p, s2p, r_sqa)

        # ---- pairwise allreduce of [128, 2] partials ----
        s12 = small.tile([128, 2], F32, tag="s12")
        nc.vector.tensor_copy(s12[:, 0:1], s1p)
        nc.vector.tensor_copy(s12[:, 1:2], s2p)
        nc.sync.dma_start(out=cc_in[:, :], in_=s12)
        if use_cc:
            nc.gpsimd.collective_compute(
                "AllReduce", mybir.AluOpType.add,
                replica_groups=[[2 * i, 2 * i + 1] for i in range(ngrp)],
                ins=[cc_in[:, :]], outs=[cc_out[:, :]])
        else:
            # diagnostic mode: no cross-core reduce (stats use only this
            # core's half; output is approximate)
            nc.sync.dma_start(out=cc_out[:, :], in_=s12)

        # ---- finish GroupNorm stats in [*, C_OUT] row layout ----
        st1 = small.tile([1, C_OUT], F32, tag="st1")
        nc.sync.dma_start(out=st1,
                          in_=bass.AP(tensor=cc_out, offset=0,
                                      ap=[[0, 1], [2, C_OUT]]))
        st2 = small.tile([1, C_OUT], F32, tag="st2")
        nc.sync.dma_start(out=st2,
                          in_=bass.AP(tensor=cc_out, offset=1,
                                      ap=[[0, 1], [2, C_OUT]]))
        sg1 = small.tile([1, G], F32, tag="sg1")
        nc.vector.tensor_reduce(out=sg1,
                                in_=st1.rearrange("p (g d) -> p g d", g=G),
                                axis=mybir.AxisListType.X,
                                op=mybir.AluOpType.add)
        sg2 = small.tile([1, G], F32, tag="sg2")
        nc.vector.tensor_reduce(out=sg2,
                                in_=st2.rearrange("p (g d) -> p g d", g=G),
                                axis=mybir.AxisListType.X,
                                op=mybir.AluOpType.add)
        mean_r = small.tile([1, G], F32, tag="mean_r")
        nc.vector.tensor_scalar(out=mean_r, in0=sg1,
                                scalar1=1.0 / gn_count, scalar2=None,
                                op0=mybir.AluOpType.mult)
        ex2_r = small.tile([1, G], F32, tag="ex2_r")
        nc.vector.tensor_scalar(out=ex2_r, in0=sg2,
                                scalar1=1.0 / gn_count, scalar2=None,
                                op0=mybir.AluOpType.mult)
        var_r = small.tile([1, G], F32, tag="var_r")
        nc.vector.tensor_tensor(out=var_r, in0=mean_r, in1=mean_r,
                                op=mybir.AluOpType.mult)
        nc.vector.tensor_tensor(out=var_r, in0=ex2_r, in1=var_r,
                                op=mybir.AluOpType.subtract)
        sd_r = small.tile([1, G], F32, tag="sd_r")
        nc.vector.tensor_scalar_add(var_r, var_r, GN_EPS)
        nc.scalar.activation(sd_r, var_r, mybir.ActivationFunctionType.Sqrt,
                             bias=0.0)
        rstd_r = small.tile([1, G], F32, tag="rstd_r")
        nc.vector.reciprocal(rstd_r, sd_r)
        mean_c = small.tile([1, C_OUT], F32, tag="mean_c")
        rstd_c = small.tile([1, C_OUT], F32, tag="rstd_c")
        gsz = C_OUT // G
        for g in range(G):
            nc.vector.tensor_copy(
                mean_c[:, g * gsz:(g + 1) * gsz],
                mean_r[:, g:g + 1].to_broadcast([1, gsz]))
            nc.vector.tensor_copy(
                rstd_c[:, g * gsz:(g + 1) * gsz],
                rstd_r[:, g:g + 1].to_broadcast([1, gsz]))
        srow = small.tile([1, C_OUT], F32, tag="srow")
        nc.vector.tensor_tensor(out=srow, in0=gam_sb, in1=rstd_c,
                                op=mybir.AluOpType.mult)
        trow = small.tile([1, C_OUT], F32, tag="trow")
        nc.vector.tensor_tensor(out=trow, in0=mean_c, in1=srow,
                                op=mybir.AluOpType.mult)
        nc.vector.tensor_tensor(out=trow, in0=bet_sb, in1=trow,
                                op=mybir.AluOpType.subtract)
        # transpose the two [1, C_OUT] rows to [C_OUT, 1] via DRAM bounce
        nc.sync.dma_start(out=row_dram[0:1, :], in_=srow)
        nc.sync.dma_start(out=row_dram[1:2, :], in_=trow)
        s_col = small.tile([C_OUT, 1], F32, tag="s_col")
        nc.sync.dma_start(out=s_col,
                          in_=bass.AP(tensor=row_dram, offset=0,
                                      ap=[[1, C_OUT], [0, 1]]))
        t_col = small.tile([C_OUT, 1], F32, tag="t_col")
        nc.sync.dma_start(out=t_col,
                          in_=bass.AP(tensor=row_dram, offset=C_OUT,
                                      ap=[[1, C_OUT], [0, 1]]))

        # ---- final normalization + relu + output ----
        for q0 in range(0, nq, qstep):
            mf = gchunk.tile([128, qstep], F32, tag="mf")
            nc.vector.tensor_add(mf, mpos_sb[:, q0:q0 + qstep],
                                 c_sb[:, q0:q0 + qstep])
            nc.vector.tensor_scalar(out=mf, in0=mf, scalar1=s_col,
                                    scalar2=t_col,
                                    op0=mybir.AluOpType.mult,
                                    op1=mybir.AluOpType.add)
            if neg_gamma:
                mn = gchunk.tile([128, qstep], F32, tag="mn")
                nc.vector.tensor_add(mn, mneg_sb[:, q0:q0 + qstep],
                                     c_sb[:, q0:q0 + qstep])
                nc.vector.tensor_scalar(out=mn, in0=mn, scalar1=s_col,
                                        scalar2=t_col,
                                        op0=mybir.AluOpType.mult,
                                        op1=mybir.AluOpType.add)
                nc.vector.tensor_tensor(out=mf, in0=mf, in1=mn,
                                        op=mybir.AluOpType.max)
            nc.vector.tensor_scalar_max(mf, mf, 0.0)
            nc.sync.dma_start(out=out_ext[:, q0:q0 + qstep], in_=mf)

    nc.finalize()
    return nc


def make_core_inputs(Fq, Fk, Pq, Pk, W, nq_half, core):
    b, h = core // 2, core % 2
    q0 = h * nq_half
    Qs = Pq[b][:, q0:q0 + nq_half]
    qt = np.concatenate([2.0 * Qs, np.ones((1, nq_half), np.float32)], 0)
    Pb = Pk[b]
    pt = np.concatenate([Pb, -(Pb * Pb).sum(0, keepdims=True)], 0)
    return {
        "qt": np.ascontiguousarray(qt, np.float32),
        "pt": np.ascontiguousarray(pt, np.float32),
        "fk": np.ascontiguousarray(Fk[b], np.float32),
        "fq": np.ascontiguousarray(Fq[b][:, q0:q0 + nq_half], np.float32),
        "w1t": np.ascontiguousarray(W[:, :C_IN].T, np.float32),
        "dt": np.ascontiguousarray((W[:, C_IN:] - W[:, :C_IN]).T, np.float32),
    }


_NC_CACHE = {}
_EXEC_CACHE = {}
TRACE = False       # kept for test.py compat; NTFF unavailable here
LAST_RESULT = None  # kept for test.py compat (always None -> wall fallback)


def _build_compiled(key, nc, n_cores=N_CORES):
    """AOT-compile the SPMD program ONCE and cache the Compiled object.

    run_bass_kernel_spmd re-creates its jit closure per call (full retrace +
    re-lower + BIR reverify every warm call) and re-fetches the global output
    once per core; this path avoids both and uses fast C++ dispatch."""
    if key in _EXEC_CACHE:
        return _EXEC_CACHE[key]
    bass2jax.install_neuronx_cc_hook()
    pname = (nc.partition_id_tensor.name
             if nc.partition_id_tensor is not None else None)
    in_names, in_avals, out_names, out_avals = [], [], [], []
    for alloc in nc.m.functions[0].allocations:
        if not isinstance(alloc, mybir.MemoryLocationSet):
            continue
        name = alloc.memorylocations[0].name
        if alloc.kind == "ExternalInput":
            if name != pname:
                in_names.append(name)
                in_avals.append((tuple(alloc.tensor_shape),
                                 mybir.dt.np(alloc.dtype)))
        elif alloc.kind == "ExternalOutput":
            out_names.append(name)
            out_avals.append(jax.core.ShapedArray(
                tuple(alloc.tensor_shape), mybir.dt.np(alloc.dtype)))
    bind_names = tuple(in_names) + ((pname,) if pname else ())

    def _body(*args):
        operands = list(args)
        if pname is not None:
            operands.append(bass2jax.partition_id_tensor())
        return tuple(bass2jax._bass_exec_p.bind(
            *operands,
            out_avals=tuple(out_avals),
            in_names=bind_names,
            out_names=tuple(out_names),
            lowering_input_output_aliases=(),
            sim_require_finite=True,
            sim_require_nnan=True,
            nc=nc,
        ))

    devices = jax.devices()[:n_cores]
    mesh = Mesh(np.asarray(devices), ("core",))
    spec = NamedSharding(mesh, PartitionSpec("core"))
    smap = jax.shard_map(
        _body, mesh=mesh,
        in_specs=(PartitionSpec("core"),) * len(in_names),
        out_specs=(PartitionSpec("core"),) * len(out_names),
        check_vma=False)
    lower_args = [
        jax.ShapeDtypeStruct((n_cores * s[0],) + s[1:], d, sharding=spec)
        for s, d in in_avals]
    compiled = bass2jax.fast_dispatch_compile(
        lambda: jax.jit(smap, keep_unused=True).lower(*lower_args).compile())
    entry = (compiled, in_names, [(s, d) for s, d in in_avals], out_names)
    _EXEC_CACHE[key] = entry
    return entry


def kernel(Fq_bcn, Fk_bcn, Pq_b3n, Pk_b3n, W_conv, gn_gamma=None,
           gn_beta=None, k=16):
    k = int(k)
    assert k == KNN, f"kernel hardcodes k=16, got {k}"
    Fq = np.asarray(Fq_bcn, np.float32)
    Fk = np.asarray(Fk_bcn, np.float32)
    Pq = np.asarray(Pq_b3n, np.float32)
    Pk = np.asarray(Pk_b3n, np.float32)
    W = np.asarray(W_conv, np.float32)
    gam = (np.ones(C_OUT, np.float32) if gn_gamma is None
           else np.asarray(gn_gamma, np.float32).reshape(C_OUT))
    bet = (np.zeros(C_OUT, np.float32) if gn_beta is None
           else np.asarray(gn_beta, np.float32).reshape(C_OUT))
    assert Fq.shape == (B, C_IN, N_KEYS)

    nq = N_KEYS // 2
    neg = bool((gam < 0).any())
    key = ("full", neg)
    if key not in _NC_CACHE:
        _NC_CACHE[key] = build_edgeconv(nq=nq, nk=N_KEYS, nseg=16,
                                        n_pair_q=N_KEYS, neg_gamma=neg)
    nc = _NC_CACHE[key]
    compiled, in_names, in_avals, out_names = _build_compiled(key, nc)

    in_maps = []
    for core in range(N_CORES):
        m = make_core_inputs(Fq, Fk, Pq, Pk, W, nq, core)
        m["gam"] = np.ascontiguousarray(gam.reshape(1, C_OUT))
        m["bet"] = np.ascontiguousarray(bet.reshape(1, C_OUT))
        in_maps.append(m)
    globals_in = [
        np.concatenate([in_maps[c][name] for c in range(N_CORES)], axis=0)
        for name in in_names]

    out_arrs = compiled(*globals_in)
    oi = out_names.index("out")
    flat = np.asarray(out_arrs[oi])  # single fetch of the global output
    out = np.empty((B, C_OUT, N_KEYS), np.float32)
    for core in range(N_CORES):
        b, h = core // 2, core % 2
        out[b, :, h * nq:(h + 1) * nq] = \
            flat[core * C_OUT:(core + 1) * C_OUT]
    return out


if __name__ == "__main__":
    rng = np.random.default_rng(0)
    inputs = {
        "Fq_bcn": rng.standard_normal((B, C_IN, N_KEYS)).astype(np.float32),
        "Fk_bcn": rng.standard_normal((B, C_IN, N_KEYS)).astype(np.float32),
        "Pq_b3n": rng.standard_normal((B, 3, N_KEYS)).astype(np.float32),
        "Pk_b3n": rng.standard_normal((B, 3, N_KEYS)).astype(np.float32),
        "W_conv": (rng.standard_normal((C_OUT, 2 * C_IN)).astype(np.float32)
                   / np.sqrt(2 * C_IN)),
        "gn_gamma": np.ones(C_OUT, np.float32),
        "gn_beta": np.zeros(C_OUT, np.float32),
        "k": 16,
    }
    out = kernel(**inputs)
    print("kernel out", out.shape, out.dtype, float(np.abs(out).mean()))



# revision 11
# speedup vs baseline: 2.9118x; 2.2048x over previous
"""EdgeConv block (kNN -> gather -> 1x1 conv -> GroupNorm -> ReLU -> max over k)
as a Bass/Tile kernel for 8 Trainium2 NeuronCores.

Problem shapes (hardcoded): B=4, C_IN=64, C_OUT=128, N=8192, K=16, G=8.

Sharding: core c handles batch b = c//2, query half h = c%2 (4096 queries).
GroupNorm statistics are partial per core and combined with a pairwise
AllReduce on a [128, 2] tensor.

Host<->device transport is the bottleneck (axon tunnel ~40MB/s with ~40ms
fixed cost per transfer), so the host interface is minimized:
  - ONE packed uint16 input tensor per core (positions f32, features fp16,
    weights fp16) -> a single ~10MB device_put for all 8 cores.
  - Key features are uploaded once per batch (each core of a pair gets half
    the key set); A = W1 @ Fk is completed on device with a pairwise
    AllGather.
  - The fp16 output is AllGathered across all 8 cores on device and only
    core 0's shard ([8*C_OUT, nq] fp16, 8MB) is fetched -- one transfer.

Math decomposition (avoids materializing [Nq, k, 2C] pair features):
  conv out[o,q,j] = W1 @ (nbr_j - Fi_q) + W2 @ Fi_q = A[o, idx[q,j]] + C[o,q]
  where A = W1 @ Fk  [O, Nk]  and  C = (W2 - W1) @ Fq  [O, Nq].
kNN scores s[q,p] = 2*Q.P - |P|^2 (monotone in -d2 per query) via fp32 PE
matmul with lhsT = [2qx; 2qy; 2qz; 1], rhs = [px; py; pz; -|P|^2].

Top-16 per query: 16 segments of Nk/16 keys; per-segment top-8 via DVE max8 +
max_index; merge the 16*8 candidates with two max8+match_replace rounds; turn
the selection mask into dense ranks with a prefix scan and compact the winning
global indices with a per-partition local_scatter.  (Exact unless >8 of the
true top-16 fall in one segment: P ~ 3e-6 per query.)

Neighbor reduction: gpsimd ap_gather of A columns (indices shared across all
128 channel partitions), then DVE blocked reduces for max_j / sum_j, fused
square-reduce for the GN second moment.

The exec path AOT-compiles the SPMD program once (cached) with fast C++
dispatch; run_bass_kernel_spmd would re-trace/re-lower every call and fetch
the global output once per core.
"""

from contextlib import ExitStack

import numpy as np
import jax
from jax.sharding import Mesh, NamedSharding, PartitionSpec

import concourse.bass as bass
import concourse.bacc as bacc
import concourse.mybir as mybir
from concourse.tile import TileContext
from concourse import bass2jax

F32 = mybir.dt.float32
F16 = mybir.dt.float16
I16 = mybir.dt.int16
U16 = mybir.dt.uint16

B, C_IN, C_OUT, N_KEYS, KNN, G = 4, 64, 128, 8192, 16, 8
GN_EPS = 1e-5
N_CORES = 8

NQ = N_KEYS // 2          # queries per core
NKH = N_KEYS // 2         # keys per core (other half arrives via AllGather)

# ---- packed input layout, offsets in uint16 units ----
OFF_QT = 0                        # f32 [4, NQ]      (2*NQ u16 per row)
OFF_PT = OFF_QT + 4 * 2 * NQ      # f32 [4, N_KEYS]
OFF_FK = OFF_PT + 4 * 2 * N_KEYS  # f16 [C_IN, NKH]
OFF_FQ = OFF_FK + C_IN * NKH      # f16 [C_IN, NQ]
OFF_W1 = OFF_FQ + C_IN * NQ       # f16 [C_IN, C_OUT]
OFF_DT = OFF_W1 + C_IN * C_OUT    # f16 [C_IN, C_OUT]
OFF_GAM = OFF_DT + C_IN * C_OUT   # f32 [1, C_OUT]
OFF_BET = OFF_GAM + 2 * C_OUT     # f32 [1, C_OUT]
TOT_U16 = OFF_BET + 2 * C_OUT


def build_edgeconv(nq=NQ, nk=N_KEYS, nseg=16, n_pair_q=N_KEYS,
                   neg_gamma=False, num_devices=N_CORES):
    """Build the SPMD program. nq: queries per core; nk: total keys per
    batch; nseg: top-k segments (seg = nk//nseg <= 512); n_pair_q: total
    queries per batch across the core pair (GN denominator)."""
    seg = nk // nseg
    assert seg * nseg == nk and seg <= 512
    ncand = nseg * 8
    qtiles = nq // 128
    assert qtiles * 128 == nq
    chunk = min(256, nq)  # queries per gather chunk
    qstep = min(512, nq)
    nchunk = nq // chunk
    assert nchunk * chunk == nq
    gn_count = float(n_pair_q * KNN * (C_OUT // G))
    ngrp = num_devices // 2
    nkh = nk // 2

    nc = bacc.Bacc("TRN2", target_bir_lowering=False, debug=False,
                   num_devices=num_devices)

    pk_ext = nc.dram_tensor("pk", [1, TOT_U16], U16, kind="ExternalInput")
    out_ext = nc.dram_tensor("out", [num_devices * C_OUT, nq], F16,
                             kind="ExternalOutput")

    idx_dram = nc.dram_tensor("idx_scratch", [nq, KNN], I16)
    row_dram = nc.dram_tensor("row_scratch", [2, C_OUT], F32)
    cc_in = nc.dram_tensor("cc_in", [C_OUT, 2], F32)
    cc_out = nc.dram_tensor("cc_out", [C_OUT, 2], F32)
    ag_in = nc.dram_tensor("ag_in", [C_OUT, nkh], F32)
    ag_out = nc.dram_tensor("ag_out", [2 * C_OUT, nkh], F32)
    og_in = nc.dram_tensor("og_in", [C_OUT, nq], F16)
    og_out = nc.dram_tensor("og_out", [num_devices * C_OUT, nq], F16)

    def pk_ap(off, rows, row_u16):
        return bass.AP(tensor=pk_ext, offset=off,
                       ap=[[row_u16, rows], [1, row_u16]])

    with TileContext(nc) as tc, ExitStack() as ctx:
        persist = ctx.enter_context(tc.tile_pool(name="persist", bufs=1))
        psum = ctx.enter_context(tc.tile_pool(name="psum", bufs=6,
                                              space="PSUM"))
        segp = ctx.enter_context(tc.tile_pool(name="segp", bufs=3))
        small = ctx.enter_context(tc.tile_pool(name="small", bufs=2))
        gchunk = ctx.enter_context(tc.tile_pool(name="gchunk", bufs=2))

        # ---- persistent SBUF, decoded from the packed input ----
        # qt/pt replicated at partition bases 0/32/64/96 so four q-tiles'
        # K=4 matmuls can run concurrently in distinct PE row groups
        rowtile = 4 if qtiles % 4 == 0 else 1
        qt_u = persist.tile([128 if rowtile == 4 else 4, 2 * nq], U16,
                            tag="qt_u")
        for r in range(rowtile):
            nc.sync.dma_start(out=qt_u[32 * r:32 * r + 4, :],
                              in_=pk_ap(OFF_QT, 4, 2 * nq))
        pt_u = persist.tile([128 if rowtile == 4 else 4, 2 * nk], U16,
                            tag="pt_u")
        for r in range(rowtile):
            nc.sync.dma_start(out=pt_u[32 * r:32 * r + 4, :],
                              in_=pk_ap(OFF_PT, 4, 2 * nk))
        w1t_u = persist.tile([C_IN, C_OUT], U16, tag="w1t_u")
        nc.sync.dma_start(out=w1t_u, in_=pk_ap(OFF_W1, C_IN, C_OUT))
        dtw_u = persist.tile([C_IN, C_OUT], U16, tag="dtw_u")
        nc.sync.dma_start(out=dtw_u, in_=pk_ap(OFF_DT, C_IN, C_OUT))
        gam_u = persist.tile([1, 2 * C_OUT], U16, tag="gam_u")
        nc.sync.dma_start(out=gam_u, in_=pk_ap(OFF_GAM, 1, 2 * C_OUT))
        bet_u = persist.tile([1, 2 * C_OUT], U16, tag="bet_u")
        nc.sync.dma_start(out=bet_u, in_=pk_ap(OFF_BET, 1, 2 * C_OUT))
        gam_sb = gam_u.bitcast(F32)
        bet_sb = bet_u.bitcast(F32)
        w1t_sb = w1t_u.bitcast(F16)
        dtw_sb = dtw_u.bitcast(F16)

        a_sb = persist.tile([C_OUT, nk], F32, tag="a_sb")
        c_sb = persist.tile([C_OUT, nq], F32, tag="c_sb")
        mpos_sb = persist.tile([C_OUT, nq], F32, tag="mpos_sb")
        mneg_sb = (persist.tile([C_OUT, nq], F32, tag="mneg_sb")
                   if neg_gamma else None)
        seg_off = persist.tile([128, ncand], I16, tag="seg_off")
        nc.gpsimd.iota(seg_off, pattern=[[seg, nseg], [0, 8]], base=0,
                       channel_multiplier=0)
        zeros_nc = persist.tile([128, ncand], F32, tag="zeros_nc")
        nc.vector.memset(zeros_nc, 0.0)

        # ---- A (half keys) and C matmuls from fp16 features; the local
        # A-half is staged in a_sb[:, :nkh] (overwritten by the gather) ----
        with tc.tile_pool(name="feat", bufs=3) as featp:
            for s0 in range(0, nkh, 512):
                fk_t = featp.tile([C_IN, 512], U16, tag="fk_t")
                nc.sync.dma_start(
                    out=fk_t,
                    in_=bass.AP(tensor=pk_ext, offset=OFF_FK + s0,
                                ap=[[nkh, C_IN], [1, 512]]))
                ps = psum.tile([C_OUT, 512], F32, tag="ps")
                nc.tensor.matmul(ps, lhsT=w1t_sb, rhs=fk_t.bitcast(F16),
                                 start=True, stop=True)
                nc.scalar.copy(out=a_sb[:, s0:s0 + 512], in_=ps)
            for s0 in range(0, nq, 512):
                fq_t = featp.tile([C_IN, 512], U16, tag="fq_t")
                nc.sync.dma_start(
                    out=fq_t,
                    in_=bass.AP(tensor=pk_ext, offset=OFF_FQ + s0,
                                ap=[[nq, C_IN], [1, 512]]))
                ps = psum.tile([C_OUT, 512], F32, tag="ps")
                nc.tensor.matmul(ps, lhsT=dtw_sb, rhs=fq_t.bitcast(F16),
                                 start=True, stop=True)
                nc.scalar.copy(out=c_sb[:, s0:s0 + 512], in_=ps)

        # complete A across the pair: each core computed its key half
        nc.sync.dma_start(out=ag_in[:, :], in_=a_sb[:, 0:nkh])
        nc.gpsimd.collective_compute(
            "AllGather", mybir.AluOpType.bypass,
            replica_groups=[[2 * i, 2 * i + 1] for i in range(ngrp)],
            ins=[ag_in[:, :]], outs=[ag_out[:, :]])
        nc.sync.dma_start(out=a_sb[:, 0:nkh], in_=ag_out[0:C_OUT, :])
        nc.sync.dma_start(out=a_sb[:, nkh:nk], in_=ag_out[C_OUT:2 * C_OUT, :])

        # stat accumulators (filled by interleaved gather chunks)
        r_sa = small.tile([128, 1], F32, tag="acc_sa")
        r_sqa = small.tile([128, 1], F32, tag="acc_sqa")
        r_csa = small.tile([128, 1], F32, tag="acc_csa")
        nc.vector.memset(r_sa, 0.0)
        nc.vector.memset(r_sqa, 0.0)
        nc.vector.memset(r_csa, 0.0)

        def emit_gather_chunk(ch):
            q0 = ch * chunk
            idxs_t = gchunk.tile([128, chunk], I16, tag="idxs_t")
            for g in range(8):
                nc.sync.dma_start(
                    out=idxs_t[g * 16:(g + 1) * 16, :],
                    in_=bass.AP(tensor=idx_dram, offset=q0 * KNN,
                                ap=[[1, KNN], [KNN, chunk]]),
                )
            ga = gchunk.tile([128, chunk * KNN], F32, tag="ga")
            nc.gpsimd.ap_gather(out_ap=ga, in_ap=a_sb, idxs_ap=idxs_t,
                                channels=128, num_elems=nk, d=1,
                                num_idxs=chunk * KNN)
            gav = ga.rearrange("p (q c) -> p q c", c=KNN)
            nc.vector.tensor_reduce(out=mpos_sb[:, q0:q0 + chunk], in_=gav,
                                    axis=mybir.AxisListType.X,
                                    op=mybir.AluOpType.max)
            if neg_gamma:
                nc.vector.tensor_reduce(out=mneg_sb[:, q0:q0 + chunk],
                                        in_=gav, axis=mybir.AxisListType.X,
                                        op=mybir.AluOpType.min)
            sa_c = gchunk.tile([128, chunk], F32, tag="sa_c")
            nc.vector.tensor_reduce(out=sa_c, in_=gav,
                                    axis=mybir.AxisListType.X,
                                    op=mybir.AluOpType.add)
            tmp1 = small.tile([128, 1], F32, tag="tmp1")
            nc.vector.tensor_reduce(out=tmp1, in_=sa_c,
                                    axis=mybir.AxisListType.X,
                                    op=mybir.AluOpType.add)
            nc.vector.tensor_add(r_sa, r_sa, tmp1)
            scr_c = gchunk.tile([128, chunk], F32, tag="scr_c")
            nc.vector.tensor_mul(scr_c, sa_c, c_sb[:, q0:q0 + chunk])
            nc.vector.tensor_reduce(out=tmp1, in_=scr_c,
                                    axis=mybir.AxisListType.X,
                                    op=mybir.AluOpType.add)
            nc.vector.tensor_add(r_csa, r_csa, tmp1)
            # in-place square on gpsimd (offloads the DVE bottleneck)
            nc.gpsimd.tensor_mul(ga, ga, ga)
            nc.vector.tensor_reduce(out=tmp1, in_=ga,
                                    axis=mybir.AxisListType.X,
                                    op=mybir.AluOpType.add)
            nc.vector.tensor_add(r_sqa, r_sqa, tmp1)

        queries_per_group = rowtile * 128
        # ---- per-q-tile kNN (row-tiled: `rowtile` q-tiles in flight),
        # with gather chunks interleaved as soon as their indices land ----
        for tq0 in range(0, qtiles, rowtile):
            cvs, cis = [], []
            for r in range(rowtile):
                cv_r = small.tile([128, ncand], F32, tag=f"cv{r}")
                ci_r = small.tile([128, ncand], U16, tag=f"ci{r}")
                cvs.append(cv_r)
                cis.append(ci_r)
            for s in range(nseg):
                for r in range(rowtile):
                    t = tq0 + r
                    lhs_q = qt_u[32 * r:32 * r + 4,
                                 256 * t:256 * (t + 1)].bitcast(F32)
                    rhs_p = pt_u[32 * r:32 * r + 4,
                                 2 * s * seg:2 * (s + 1) * seg].bitcast(F32)
                    ps = psum.tile([128, seg], F32, tag="ps")
                    nc.tensor.matmul(ps, lhsT=lhs_q, rhs=rhs_p,
                                     start=True, stop=True,
                                     tile_position=(32 * r, 0))
                    ssb = segp.tile([128, seg], F32, tag="ssb")
                    nc.scalar.copy(out=ssb, in_=ps)
                    nc.vector.max(out=cvs[r][:, s * 8:(s + 1) * 8], in_=ssb)
                    nc.vector.max_index(out=cis[r][:, s * 8:(s + 1) * 8],
                                        in_max=cvs[r][:, s * 8:(s + 1) * 8],
                                        in_values=ssb)
            for r in range(rowtile):
                t = tq0 + r
                cv, ci = cvs[r], cis[r]
                v1 = small.tile([128, 8], F32, tag="v1")
                v2 = small.tile([128, 8], F32, tag="v2")
                cv2 = small.tile([128, ncand], F32, tag="cv2")
                cv3 = small.tile([128, ncand], F32, tag="cv3")
                nc.vector.max(out=v1, in_=cv)
                nc.vector.match_replace(out=cv2, in_to_replace=v1,
                                        in_values=cv, imm_value=-1e30)
                nc.vector.max(out=v2, in_=cv2)
                nc.vector.match_replace(out=cv3, in_to_replace=v2,
                                        in_values=cv2, imm_value=-1e30)
                maskf = small.tile([128, ncand], F32, tag="maskf")
                nc.vector.tensor_tensor(out=maskf, in0=cv, in1=cv3,
                                        op=mybir.AluOpType.not_equal)
                rk = small.tile([128, ncand], F32, tag="rk")
                nc.vector.tensor_tensor_scan(out=rk, data0=maskf,
                                             data1=zeros_nc, initial=0.0,
                                             op0=mybir.AluOpType.add,
                                             op1=mybir.AluOpType.add)
                tgt = small.tile([128, ncand], F32, tag="tgt")
                nc.vector.tensor_tensor(out=tgt, in0=rk, in1=maskf,
                                        op=mybir.AluOpType.mult)
                nc.vector.tensor_scalar_add(tgt, tgt, -1.0)
                tgt_i = small.tile([128, ncand], I16, tag="tgti")
                nc.vector.tensor_copy(tgt_i, tgt)
                gidx = small.tile([128, ncand], I16, tag="gidx")
                nc.vector.tensor_tensor(out=gidx, in0=ci.bitcast(I16),
                                        in1=seg_off, op=mybir.AluOpType.add)
                idx16 = small.tile([128, KNN], I16, tag="idx16")
                nc.gpsimd.local_scatter(out_ap=idx16, data_ap=gidx,
                                        idxs_ap=tgt_i, channels=128,
                                        num_elems=KNN, num_idxs=ncand)
                nc.sync.dma_start(out=idx_dram[t * 128:(t + 1) * 128, :],
                                  in_=idx16)
            # emit the PREVIOUS group's gather chunks: their idx writes
            # have had a full group of kNN work to complete, so the DRAM
            # round-trip latency is hidden
            if tq0 > 0:
                prev_q0 = (tq0 - rowtile) * 128
                for ch in range(prev_q0 // chunk,
                                (prev_q0 + queries_per_group) // chunk):
                    emit_gather_chunk(ch)

        # flush the final group's gather chunks
        last_q0 = (qtiles - rowtile) * 128
        for ch in range(last_q0 // chunk,
                        (last_q0 + queries_per_group) // chunk):
            emit_gather_chunk(ch)

        r_c = small.tile([128, 1], F32, tag="r_c")
        nc.vector.tensor_reduce(out=r_c, in_=c_sb,
                                axis=mybir.AxisListType.X,
                                op=mybir.AluOpType.add)
        r_c2 = small.tile([128, 1], F32, tag="r_c2")
        nc.vector.memset(r_c2, 0.0)
        tmpc = small.tile([128, 1], F32, tag="tmpc")
        for q0 in range(0, nq, qstep):
            scr5 = gchunk.tile([128, qstep], F32, tag="scr5")
            nc.vector.tensor_mul(scr5, c_sb[:, q0:q0 + qstep],
                                 c_sb[:, q0:q0 + qstep])
            nc.vector.tensor_reduce(out=tmpc, in_=scr5,
                                    axis=mybir.AxisListType.X,
                                    op=mybir.AluOpType.add)
            nc.vector.tensor_add(r_c2, r_c2, tmpc)

        s1p = small.tile([128, 1], F32, tag="s1p")
        nc.vector.tensor_scalar(out=s1p, in0=r_c, scalar1=float(KNN),
                                scalar2=None, op0=mybir.AluOpType.mult)
        nc.vector.tensor_add(s1p, s1p, r_sa)
        s2p = small.tile([128, 1], F32, tag="s2p")
        nc.vector.tensor_scalar(out=s2p, in0=r_c2, scalar1=float(KNN),
                                scalar2=None, op0=mybir.AluOpType.mult)
        t2 = small.tile([128, 1], F32, tag="t2")
        nc.vector.tensor_scalar(out=t2, in0=r_csa, scalar1=2.0,
                                scalar2=None, op0=mybir.AluOpType.mult)
        nc.vector.tensor_add(s2p, s2p, t2)
        nc.vector.tensor_add(s2p, s2p, r_sqa)

        # ---- pairwise allreduce of [128, 2] partials ----
        s12 = small.tile([128, 2], F32, tag="s12")
        nc.vector.tensor_copy(s12[:, 0:1], s1p)
        nc.vector.tensor_copy(s12[:, 1:2], s2p)
        nc.sync.dma_start(out=cc_in[:, :], in_=s12)
        nc.gpsimd.collective_compute(
            "AllReduce", mybir.AluOpType.add,
            replica_groups=[[2 * i, 2 * i + 1] for i in range(ngrp)],
            ins=[cc_in[:, :]], outs=[cc_out[:, :]])

        # ---- finish GroupNorm stats in [*, C_OUT] row layout ----
        st1 = small.tile([1, C_OUT], F32, tag="st1")
        nc.sync.dma_start(out=st1,
                          in_=bass.AP(tensor=cc_out, offset=0,
                                      ap=[[0, 1], [2, C_OUT]]))
        st2 = small.tile([1, C_OUT], F32, tag="st2")
        nc.sync.dma_start(out=st2,
                          in_=bass.AP(tensor=cc_out, offset=1,
                                      ap=[[0, 1], [2, C_OUT]]))
        sg1 = small.tile([1, G], F32, tag="sg1")
        nc.vector.tensor_reduce(out=sg1,
                                in_=st1.rearrange("p (g d) -> p g d", g=G),
                                axis=mybir.AxisListType.X,
                                op=mybir.AluOpType.add)
        sg2 = small.tile([1, G], F32, tag="sg2")
        nc.vector.tensor_reduce(out=sg2,
                                in_=st2.rearrange("p (g d) -> p g d", g=G),
                                axis=mybir.AxisListType.X,
                                op=mybir.AluOpType.add)
        mean_r = small.tile([1, G], F32, tag="mean_r")
        nc.vector.tensor_scalar(out=mean_r, in0=sg1,
                                scalar1=1.0 / gn_count, scalar2=None,
                                op0=mybir.AluOpType.mult)
        ex2_r = small.tile([1, G], F32, tag="ex2_r")
        nc.vector.tensor_scalar(out=ex2_r, in0=sg2,
                                scalar1=1.0 / gn_count, scalar2=None,
                                op0=mybir.AluOpType.mult)
        var_r = small.tile([1, G], F32, tag="var_r")
        nc.vector.tensor_tensor(out=var_r, in0=mean_r, in1=mean_r,
                                op=mybir.AluOpType.mult)
        nc.vector.tensor_tensor(out=var_r, in0=ex2_r, in1=var_r,
                                op=mybir.AluOpType.subtract)
        sd_r = small.tile([1, G], F32, tag="sd_r")
        nc.vector.tensor_scalar_add(var_r, var_r, GN_EPS)
        nc.scalar.activation(sd_r, var_r, mybir.ActivationFunctionType.Sqrt,
                             bias=0.0)
        rstd_r = small.tile([1, G], F32, tag="rstd_r")
        nc.vector.reciprocal(rstd_r, sd_r)
        mean_c = small.tile([1, C_OUT], F32, tag="mean_c")
        rstd_c = small.tile([1, C_OUT], F32, tag="rstd_c")
        gsz = C_OUT // G
        for g in range(G):
            nc.vector.tensor_copy(
                mean_c[:, g * gsz:(g + 1) * gsz],
                mean_r[:, g:g + 1].to_broadcast([1, gsz]))
            nc.vector.tensor_copy(
                rstd_c[:, g * gsz:(g + 1) * gsz],
                rstd_r[:, g:g + 1].to_broadcast([1, gsz]))
        srow = small.tile([1, C_OUT], F32, tag="srow")
        nc.vector.tensor_tensor(out=srow, in0=gam_sb, in1=rstd_c,
                                op=mybir.AluOpType.mult)
        trow = small.tile([1, C_OUT], F32, tag="trow")
        nc.vector.tensor_tensor(out=trow, in0=mean_c, in1=srow,
                                op=mybir.AluOpType.mult)
        nc.vector.tensor_tensor(out=trow, in0=bet_sb, in1=trow,
                                op=mybir.AluOpType.subtract)
        # transpose the two [1, C_OUT] rows to [C_OUT, 1] via DRAM bounce
        nc.sync.dma_start(out=row_dram[0:1, :], in_=srow)
        nc.sync.dma_start(out=row_dram[1:2, :], in_=trow)
        s_col = small.tile([C_OUT, 1], F32, tag="s_col")
        nc.sync.dma_start(out=s_col,
                          in_=bass.AP(tensor=row_dram, offset=0,
                                      ap=[[1, C_OUT], [0, 1]]))
        t_col = small.tile([C_OUT, 1], F32, tag="t_col")
        nc.sync.dma_start(out=t_col,
                          in_=bass.AP(tensor=row_dram, offset=C_OUT,
                                      ap=[[1, C_OUT], [0, 1]]))

        # ---- final normalization + relu -> fp16, staged for out-gather ----
        for q0 in range(0, nq, qstep):
            mf = gchunk.tile([128, qstep], F32, tag="mf")
            nc.vector.tensor_add(mf, mpos_sb[:, q0:q0 + qstep],
                                 c_sb[:, q0:q0 + qstep])
            nc.vector.tensor_scalar(out=mf, in0=mf, scalar1=s_col,
                                    scalar2=t_col,
                                    op0=mybir.AluOpType.mult,
                                    op1=mybir.AluOpType.add)
            if neg_gamma:
                mn = gchunk.tile([128, qstep], F32, tag="mn")
                nc.vector.tensor_add(mn, mneg_sb[:, q0:q0 + qstep],
                                     c_sb[:, q0:q0 + qstep])
                nc.vector.tensor_scalar(out=mn, in0=mn, scalar1=s_col,
                                        scalar2=t_col,
                                        op0=mybir.AluOpType.mult,
                                        op1=mybir.AluOpType.add)
                nc.vector.tensor_tensor(out=mf, in0=mf, in1=mn,
                                        op=mybir.AluOpType.max)
            nc.vector.tensor_scalar_max(mf, mf, 0.0)
            mh = gchunk.tile([128, qstep], F16, tag="mh")
            nc.vector.tensor_copy(mh, mf)
            nc.sync.dma_start(out=og_in[:, q0:q0 + qstep], in_=mh)

        # gather every core's [C_OUT, nq] fp16 block onto all cores; the
        # host fetches only core 0's shard (one 8MB transfer). The
        # verifier forbids collectives writing IO tensors, so bounce
        # through internal DRAM.
        nc.gpsimd.collective_compute(
            "AllGather", mybir.AluOpType.bypass,
            replica_groups=[list(range(num_devices))],
            ins=[og_in[:, :]], outs=[og_out[:, :]])
        nc.sync.dma_start(out=out_ext[:, :], in_=og_out[:, :])

    nc.finalize()
    return nc


def pack_inputs(Fq, Fk, Pq, Pk, W, gam, bet):
    """Pack all per-core inputs into one [N_CORES, TOT_U16] uint16 array."""
    w1 = W[:, :C_IN]
    w1t16 = np.ascontiguousarray(w1.T, np.float16).view(np.uint16)
    dt16 = np.ascontiguousarray((W[:, C_IN:] - w1).T,
                                np.float16).view(np.uint16)
    gam16 = np.ascontiguousarray(gam, np.float32).view(np.uint16)
    bet16 = np.ascontiguousarray(bet, np.float32).view(np.uint16)
    buf = np.empty((N_CORES, TOT_U16), np.uint16)
    for core in range(N_CORES):
        b, h = core // 2, core % 2
        q0 = h * NQ
        row = buf[core]
        Qs = Pq[b][:, q0:q0 + NQ]
        qt = np.concatenate([2.0 * Qs, np.ones((1, NQ), np.float32)], 0)
        row[OFF_QT:OFF_PT] = np.ascontiguousarray(qt, np.float32) \
            .view(np.uint16).ravel()
        Pb = Pk[b]
        pt = np.concatenate([Pb, -(Pb * Pb).sum(0, keepdims=True)], 0)
        row[OFF_PT:OFF_FK] = np.ascontiguousarray(pt, np.float32) \
            .view(np.uint16).ravel()
        row[OFF_FK:OFF_FQ] = np.ascontiguousarray(
            Fk[b][:, h * NKH:(h + 1) * NKH], np.float16).view(np.uint16) \
            .ravel()
        row[OFF_FQ:OFF_W1] = np.ascontiguousarray(
            Fq[b][:, q0:q0 + NQ], np.float16).view(np.uint16).ravel()
        row[OFF_W1:OFF_DT] = w1t16.ravel()
        row[OFF_DT:OFF_GAM] = dt16.ravel()
        row[OFF_GAM:OFF_BET] = gam16.ravel()
        row[OFF_BET:TOT_U16] = bet16.ravel()
    return buf


_NC_CACHE = {}
_EXEC_CACHE = {}
TRACE = False       # kept for test.py compat; NTFF unavailable here
LAST_RESULT = None  # kept for test.py compat (always None -> wall fallback)


def _build_compiled(key, nc, n_cores=N_CORES):
    """AOT-compile the SPMD program ONCE and cache the Compiled object."""
    if key in _EXEC_CACHE:
        return _EXEC_CACHE[key]
    bass2jax.install_neuronx_cc_hook()
    pname = (nc.partition_id_tensor.name
             if nc.partition_id_tensor is not None else None)
    in_names, in_avals, out_names, out_avals = [], [], [], []
    for alloc in nc.m.functions[0].allocations:
        if not isinstance(alloc, mybir.MemoryLocationSet):
            continue
        name = alloc.memorylocations[0].name
        if alloc.kind == "ExternalInput":
            if name != pname:
                in_names.append(name)
                in_avals.append((tuple(alloc.tensor_shape),
                                 mybir.dt.np(alloc.dtype)))
        elif alloc.kind == "ExternalOutput":
            out_names.append(name)
            out_avals.append(jax.core.ShapedArray(
                tuple(alloc.tensor_shape), mybir.dt.np(alloc.dtype)))
    bind_names = tuple(in_names) + ((pname,) if pname else ())

    def _body(*args):
        operands = list(args)
        if pname is not None:
            operands.append(bass2jax.partition_id_tensor())
        return tuple(bass2jax._bass_exec_p.bind(
            *operands,
            out_avals=tuple(out_avals),
            in_names=bind_names,
            out_names=tuple(out_names),
            lowering_input_output_aliases=(),
            sim_require_finite=True,
            sim_require_nnan=True,
            nc=nc,
        ))

    devices = jax.devices()[:n_cores]
    mesh = Mesh(np.asarray(devices), ("core",))
    spec = NamedSharding(mesh, PartitionSpec("core"))
    smap = jax.shard_map(
        _body, mesh=mesh,
        in_specs=(PartitionSpec("core"),) * len(in_names),
        out_specs=(PartitionSpec("core"),) * len(out_names),
        check_vma=False)
    lower_args = [
        jax.ShapeDtypeStruct((n_cores * s[0],) + s[1:], d, sharding=spec)
        for s, d in in_avals]
    compiled = bass2jax.fast_dispatch_compile(
        lambda: jax.jit(smap, keep_unused=True).lower(*lower_args).compile())
    entry = (compiled, in_names, out_names)
    _EXEC_CACHE[key] = entry
    return entry


def kernel(Fq_bcn, Fk_bcn, Pq_b3n, Pk_b3n, W_conv, gn_gamma=None,
           gn_beta=None, k=16):
    k = int(k)
    assert k == KNN, f"kernel hardcodes k=16, got {k}"
    Fq = np.asarray(Fq_bcn, np.float32)
    Fk = np.asarray(Fk_bcn, np.float32)
    Pq = np.asarray(Pq_b3n, np.float32)
    Pk = np.asarray(Pk_b3n, np.float32)
    W = np.asarray(W_conv, np.float32)
    gam = (np.ones(C_OUT, np.float32) if gn_gamma is None
           else np.asarray(gn_gamma, np.float32).reshape(C_OUT))
    bet = (np.zeros(C_OUT, np.float32) if gn_beta is None
           else np.asarray(gn_beta, np.float32).reshape(C_OUT))
    assert Fq.shape == (B, C_IN, N_KEYS)

    neg = bool((gam < 0).any())
    key = ("packed", neg)
    if key not in _NC_CACHE:
        _NC_CACHE[key] = build_edgeconv(neg_gamma=neg)
    nc = _NC_CACHE[key]
    compiled, in_names, out_names = _build_compiled(key, nc)

    packed = pack_inputs(Fq, Fk, Pq, Pk, W, gam, bet)
    out_arrs = compiled(packed)
    gathered = out_arrs[out_names.index("out")]
    # only core 0's shard is materialized: [N_CORES*C_OUT, NQ] fp16
    shard0 = np.asarray(gathered.addressable_shards[0].data)
    out = np.empty((B, C_OUT, N_KEYS), np.float32)
    for core in range(N_CORES):
        b, h = core // 2, core % 2
        out[b, :, h * NQ:(h + 1) * NQ] = \
            shard0[core * C_OUT:(core + 1) * C_OUT]
    return out


if __name__ == "__main__":
    rng = np.random.default_rng(0)
    inputs = {
        "Fq_bcn": rng.standard_normal((B, C_IN, N_KEYS)).astype(np.float32),
        "Fk_bcn": rng.standard_normal((B, C_IN, N_KEYS)).astype(np.float32),
        "Pq_b3n": rng.standard_normal((B, 3, N_KEYS)).astype(np.float32),
        "Pk_b3n": rng.standard_normal((B, 3, N_KEYS)).astype(np.float32),
        "W_conv": (rng.standard_normal((C_OUT, 2 * C_IN)).astype(np.float32)
                   / np.sqrt(2 * C_IN)),
        "gn_gamma": np.ones(C_OUT, np.float32),
        "gn_beta": np.zeros(C_OUT, np.float32),
        "k": 16,
    }
    out = kernel(**inputs)
    print("kernel out", out.shape, out.dtype, float(np.abs(out).mean()))


# revision 18
# speedup vs baseline: 3.9554x; 1.3584x over previous
"""EdgeConv block (kNN -> gather -> 1x1 conv -> GroupNorm -> ReLU -> max over k)
as a Bass/Tile kernel for 8 Trainium2 NeuronCores.

Problem shapes (hardcoded): B=4, C_IN=64, C_OUT=128, N=8192, K=16, G=8.

Sharding: core c handles batch b = c//2, query half h = c%2 (4096 queries).
GroupNorm statistics are partial per core and combined with a pairwise
AllReduce on a [128, 2] tensor.

Host<->device transport is the bottleneck (axon tunnel ~40MB/s with ~40ms
fixed cost per transfer), so the host interface is minimized:
  - ONE packed uint16 input tensor per core (positions f32, features fp16,
    weights fp16) -> a single ~10MB device_put for all 8 cores.
  - Key features are uploaded once per batch (each core of a pair gets half
    the key set); A = W1 @ Fk is completed on device with a pairwise
    AllGather.
  - The fp16 output is AllGathered across all 8 cores on device and only
    core 0's shard ([8*C_OUT, nq] fp16, 8MB) is fetched -- one transfer.

Math decomposition (avoids materializing [Nq, k, 2C] pair features):
  conv out[o,q,j] = W1 @ (nbr_j - Fi_q) + W2 @ Fi_q = A[o, idx[q,j]] + C[o,q]
  where A = W1 @ Fk  [O, Nk]  and  C = (W2 - W1) @ Fq  [O, Nq].
kNN scores s[q,p] = 2*Q.P - |P|^2 (monotone in -d2 per query) via fp32 PE
matmul with lhsT = [2qx; 2qy; 2qz; 1], rhs = [px; py; pz; -|P|^2].

Top-16 per query: 16 segments of Nk/16 keys; per-segment top-8 via DVE max8 +
max_index; merge the 16*8 candidates with two max8+match_replace rounds; turn
the selection mask into dense ranks with a prefix scan and compact the winning
global indices with a per-partition local_scatter.  (Exact unless >8 of the
true top-16 fall in one segment: P ~ 3e-6 per query.)

Neighbor reduction: gpsimd ap_gather of A columns (indices shared across all
128 channel partitions), then DVE blocked reduces for max_j / sum_j, fused
square-reduce for the GN second moment.

The exec path AOT-compiles the SPMD program once (cached) with fast C++
dispatch; run_bass_kernel_spmd would re-trace/re-lower every call and fetch
the global output once per core.
"""

from contextlib import ExitStack

import numpy as np
import jax
from jax.sharding import Mesh, NamedSharding, PartitionSpec

import concourse.bass as bass
import concourse.bacc as bacc
import concourse.mybir as mybir
from concourse.tile import TileContext
from concourse import bass2jax

F32 = mybir.dt.float32
F16 = mybir.dt.float16
I16 = mybir.dt.int16
U16 = mybir.dt.uint16
U8 = mybir.dt.uint8

B, C_IN, C_OUT, N_KEYS, KNN, G = 4, 64, 128, 8192, 16, 8
GN_EPS = 1e-5
N_CORES = 8

NQ = N_KEYS // 2          # queries per core
NKH = N_KEYS // 2         # keys per core (other half arrives via AllGather)

# ---- packed input layout, offsets in uint16 units ----
OFF_QT = 0                        # f32 [4, NQ]      (2*NQ u16 per row)
OFF_PT = OFF_QT + 4 * 2 * NQ      # f32 [4, N_KEYS]
OFF_FK = OFF_PT + 4 * 2 * N_KEYS  # f16 [C_IN, NKH]
OFF_FQ = OFF_FK + C_IN * NKH      # f16 [C_IN, NQ]
OFF_W1 = OFF_FQ + C_IN * NQ       # f16 [C_IN, C_OUT]
OFF_DT = OFF_W1 + C_IN * C_OUT    # f16 [C_IN, C_OUT]
OFF_GAM = OFF_DT + C_IN * C_OUT   # f32 [1, C_OUT]
OFF_BET = OFF_GAM + 2 * C_OUT     # f32 [1, C_OUT]
TOT_U16 = OFF_BET + 2 * C_OUT


def build_edgeconv(nq=NQ, nk=N_KEYS, nseg=16, n_pair_q=N_KEYS,
                   neg_gamma=False, num_devices=N_CORES, qscale=42.5):
    """Build the SPMD program. nq: queries per core; nk: total keys per
    batch; nseg: top-k segments (seg = nk//nseg <= 512); n_pair_q: total
    queries per batch across the core pair (GN denominator)."""
    seg = nk // nseg
    assert seg * nseg == nk and seg <= 512
    ncand = nseg * 8
    qtiles = nq // 128
    assert qtiles * 128 == nq
    chunk = min(256, nq)  # queries per gather chunk
    qstep = min(512, nq)
    nchunk = nq // chunk
    assert nchunk * chunk == nq
    gn_count = float(n_pair_q * KNN * (C_OUT // G))
    ngrp = num_devices // 2
    nkh = nk // 2

    nc = bacc.Bacc("TRN2", target_bir_lowering=False, debug=False,
                   num_devices=num_devices)

    pk_ext = nc.dram_tensor("pk", [1, TOT_U16], U16, kind="ExternalInput")
    out_ext = nc.dram_tensor("out", [num_devices * C_OUT, nq], U8,
                             kind="ExternalOutput")

    idx_dram = nc.dram_tensor("idx_scratch", [nq, KNN], I16)
    row_dram = nc.dram_tensor("row_scratch", [2, C_OUT], F32)
    cc_in = nc.dram_tensor("cc_in", [C_OUT, 2], F32)
    cc_out = nc.dram_tensor("cc_out", [C_OUT, 2], F32)
    ag_in = nc.dram_tensor("ag_in", [C_OUT, nkh], F32)
    ag_out = nc.dram_tensor("ag_out", [2 * C_OUT, nkh], F32)
    og_in = nc.dram_tensor("og_in", [C_OUT, nq], U8)
    og_out = nc.dram_tensor("og_out", [num_devices * C_OUT, nq], U8,
                            addr_space="Shared")

    def pk_ap(off, rows, row_u16):
        return bass.AP(tensor=pk_ext, offset=off,
                       ap=[[row_u16, rows], [1, row_u16]])

    with TileContext(nc) as tc, ExitStack() as ctx:
        persist = ctx.enter_context(tc.tile_pool(name="persist", bufs=1))
        psum = ctx.enter_context(tc.tile_pool(name="psum", bufs=6,
                                              space="PSUM"))
        segp = ctx.enter_context(tc.tile_pool(name="segp", bufs=3))
        small = ctx.enter_context(tc.tile_pool(name="small", bufs=2))
        gchunk = ctx.enter_context(tc.tile_pool(name="gchunk", bufs=2))

        # ---- persistent SBUF, decoded from the packed input ----
        # qt/pt replicated at partition bases 0/32/64/96 so four q-tiles'
        # K=4 matmuls can run concurrently in distinct PE row groups
        rowtile = 4 if qtiles % 4 == 0 else 1
        qt_u = persist.tile([128 if rowtile == 4 else 4, 2 * nq], U16,
                            tag="qt_u")
        for r in range(rowtile):
            nc.sync.dma_start(out=qt_u[32 * r:32 * r + 4, :],
                              in_=pk_ap(OFF_QT, 4, 2 * nq))
        pt_u = persist.tile([128 if rowtile == 4 else 4, 2 * nk], U16,
                            tag="pt_u")
        for r in range(rowtile):
            nc.sync.dma_start(out=pt_u[32 * r:32 * r + 4, :],
                              in_=pk_ap(OFF_PT, 4, 2 * nk))
        w1t_u = persist.tile([C_IN, C_OUT], U16, tag="w1t_u")
        nc.sync.dma_start(out=w1t_u, in_=pk_ap(OFF_W1, C_IN, C_OUT))
        dtw_u = persist.tile([C_IN, C_OUT], U16, tag="dtw_u")
        nc.sync.dma_start(out=dtw_u, in_=pk_ap(OFF_DT, C_IN, C_OUT))
        gam_u = persist.tile([1, 2 * C_OUT], U16, tag="gam_u")
        nc.sync.dma_start(out=gam_u, in_=pk_ap(OFF_GAM, 1, 2 * C_OUT))
        bet_u = persist.tile([1, 2 * C_OUT], U16, tag="bet_u")
        nc.sync.dma_start(out=bet_u, in_=pk_ap(OFF_BET, 1, 2 * C_OUT))
        gam_sb = gam_u.bitcast(F32)
        bet_sb = bet_u.bitcast(F32)
        w1t_sb = w1t_u.bitcast(F16)
        dtw_sb = dtw_u.bitcast(F16)

        a_sb = persist.tile([C_OUT, nk], F32, tag="a_sb")
        c_sb = persist.tile([C_OUT, nq], F32, tag="c_sb")
        mpos_sb = persist.tile([C_OUT, nq], F32, tag="mpos_sb")
        mneg_sb = (persist.tile([C_OUT, nq], F32, tag="mneg_sb")
                   if neg_gamma else None)
        seg_off = persist.tile([128, ncand], I16, tag="seg_off")
        nc.gpsimd.iota(seg_off, pattern=[[seg, nseg], [0, 8]], base=0,
                       channel_multiplier=0)
        zeros_nc = persist.tile([128, ncand], F32, tag="zeros_nc")
        nc.vector.memset(zeros_nc, 0.0)

        # ---- A (half keys) and C matmuls from fp16 features; the local
        # A-half is staged in a_sb[:, :nkh] (overwritten by the gather) ----
        with tc.tile_pool(name="feat", bufs=3) as featp:
            for s0 in range(0, nkh, 512):
                fk_t = featp.tile([C_IN, 512], U16, tag="fk_t")
                nc.sync.dma_start(
                    out=fk_t,
                    in_=bass.AP(tensor=pk_ext, offset=OFF_FK + s0,
                                ap=[[nkh, C_IN], [1, 512]]))
                ps = psum.tile([C_OUT, 512], F32, tag="ps")
                nc.tensor.matmul(ps, lhsT=w1t_sb, rhs=fk_t.bitcast(F16),
                                 start=True, stop=True)
                nc.scalar.copy(out=a_sb[:, s0:s0 + 512], in_=ps)
            for s0 in range(0, nq, 512):
                fq_t = featp.tile([C_IN, 512], U16, tag="fq_t")
                nc.sync.dma_start(
                    out=fq_t,
                    in_=bass.AP(tensor=pk_ext, offset=OFF_FQ + s0,
                                ap=[[nq, C_IN], [1, 512]]))
                ps = psum.tile([C_OUT, 512], F32, tag="ps")
                nc.tensor.matmul(ps, lhsT=dtw_sb, rhs=fq_t.bitcast(F16),
                                 start=True, stop=True)
                nc.scalar.copy(out=c_sb[:, s0:s0 + 512], in_=ps)

        # complete A across the pair: each core computed its key half
        nc.sync.dma_start(out=ag_in[:, :], in_=a_sb[:, 0:nkh])
        nc.gpsimd.collective_compute(
            "AllGather", mybir.AluOpType.bypass,
            replica_groups=[[2 * i, 2 * i + 1] for i in range(ngrp)],
            ins=[ag_in[:, :]], outs=[ag_out[:, :]])
        nc.sync.dma_start(out=a_sb[:, 0:nkh], in_=ag_out[0:C_OUT, :])
        nc.sync.dma_start(out=a_sb[:, nkh:nk], in_=ag_out[C_OUT:2 * C_OUT, :])

        # stat accumulators (filled by interleaved gather chunks)
        r_sa = small.tile([128, 1], F32, tag="acc_sa")
        r_sqa = small.tile([128, 1], F32, tag="acc_sqa")
        r_csa = small.tile([128, 1], F32, tag="acc_csa")
        nc.vector.memset(r_sa, 0.0)
        nc.vector.memset(r_sqa, 0.0)
        nc.vector.memset(r_csa, 0.0)

        def emit_gather_chunk(ch):
            q0 = ch * chunk
            idxs_t = gchunk.tile([128, chunk], I16, tag="idxs_t")
            for g in range(8):
                nc.sync.dma_start(
                    out=idxs_t[g * 16:(g + 1) * 16, :],
                    in_=bass.AP(tensor=idx_dram, offset=q0 * KNN,
                                ap=[[1, KNN], [KNN, chunk]]),
                )
            ga = gchunk.tile([128, chunk * KNN], F32, tag="ga")
            nc.gpsimd.ap_gather(out_ap=ga, in_ap=a_sb, idxs_ap=idxs_t,
                                channels=128, num_elems=nk, d=1,
                                num_idxs=chunk * KNN)
            gav = ga.rearrange("p (q c) -> p q c", c=KNN)
            nc.vector.tensor_reduce(out=mpos_sb[:, q0:q0 + chunk], in_=gav,
                                    axis=mybir.AxisListType.X,
                                    op=mybir.AluOpType.max)
            if neg_gamma:
                nc.vector.tensor_reduce(out=mneg_sb[:, q0:q0 + chunk],
                                        in_=gav, axis=mybir.AxisListType.X,
                                        op=mybir.AluOpType.min)
            sa_c = gchunk.tile([128, chunk], F32, tag="sa_c")
            nc.vector.tensor_reduce(out=sa_c, in_=gav,
                                    axis=mybir.AxisListType.X,
                                    op=mybir.AluOpType.add)
            tmp1 = small.tile([128, 1], F32, tag="tmp1")
            nc.vector.tensor_reduce(out=tmp1, in_=sa_c,
                                    axis=mybir.AxisListType.X,
                                    op=mybir.AluOpType.add)
            nc.vector.tensor_add(r_sa, r_sa, tmp1)
            scr_c = gchunk.tile([128, chunk], F32, tag="scr_c")
            nc.vector.tensor_mul(scr_c, sa_c, c_sb[:, q0:q0 + chunk])
            nc.vector.tensor_reduce(out=tmp1, in_=scr_c,
                                    axis=mybir.AxisListType.X,
                                    op=mybir.AluOpType.add)
            nc.vector.tensor_add(r_csa, r_csa, tmp1)
            # in-place square on gpsimd (offloads the DVE bottleneck)
            nc.gpsimd.tensor_mul(ga, ga, ga)
            nc.vector.tensor_reduce(out=tmp1, in_=ga,
                                    axis=mybir.AxisListType.X,
                                    op=mybir.AluOpType.add)
            nc.vector.tensor_add(r_sqa, r_sqa, tmp1)

        queries_per_group = rowtile * 128
        # ---- per-q-tile kNN (row-tiled: `rowtile` q-tiles in flight),
        # with gather chunks interleaved as soon as their indices land ----
        for tq0 in range(0, qtiles, rowtile):
            cvs, cis = [], []
            for r in range(rowtile):
                cv_r = small.tile([128, ncand], F32, tag=f"cv{r}")
                ci_r = small.tile([128, ncand], U16, tag=f"ci{r}")
                cvs.append(cv_r)
                cis.append(ci_r)
            for s in range(nseg):
                for r in range(rowtile):
                    t = tq0 + r
                    lhs_q = qt_u[32 * r:32 * r + 4,
                                 256 * t:256 * (t + 1)].bitcast(F32)
                    rhs_p = pt_u[32 * r:32 * r + 4,
                                 2 * s * seg:2 * (s + 1) * seg].bitcast(F32)
                    ps = psum.tile([128, seg], F32, tag="ps")
                    nc.tensor.matmul(ps, lhsT=lhs_q, rhs=rhs_p,
                                     start=True, stop=True,
                                     tile_position=(32 * r, 0))
                    ssb = segp.tile([128, seg], F32, tag="ssb")
                    nc.scalar.copy(out=ssb, in_=ps)
                    nc.vector.max(out=cvs[r][:, s * 8:(s + 1) * 8], in_=ssb)
                    nc.vector.max_index(out=cis[r][:, s * 8:(s + 1) * 8],
                                        in_max=cvs[r][:, s * 8:(s + 1) * 8],
                                        in_values=ssb)
            for r in range(rowtile):
                t = tq0 + r
                cv, ci = cvs[r], cis[r]
                v1 = small.tile([128, 8], F32, tag="v1")
                v2 = small.tile([128, 8], F32, tag="v2")
                cv2 = small.tile([128, ncand], F32, tag="cv2")
                cv3 = small.tile([128, ncand], F32, tag="cv3")
                nc.vector.max(out=v1, in_=cv)
                nc.vector.match_replace(out=cv2, in_to_replace=v1,
                                        in_values=cv, imm_value=-1e30)
                nc.vector.max(out=v2, in_=cv2)
                nc.vector.match_replace(out=cv3, in_to_replace=v2,
                                        in_values=cv2, imm_value=-1e30)
                maskf = small.tile([128, ncand], F32, tag="maskf")
                nc.vector.tensor_tensor(out=maskf, in0=cv, in1=cv3,
                                        op=mybir.AluOpType.not_equal)
                rk = small.tile([128, ncand], F32, tag="rk")
                nc.vector.tensor_tensor_scan(out=rk, data0=maskf,
                                             data1=zeros_nc, initial=0.0,
                                             op0=mybir.AluOpType.add,
                                             op1=mybir.AluOpType.add)
                tgt = small.tile([128, ncand], F32, tag="tgt")
                nc.vector.tensor_tensor(out=tgt, in0=rk, in1=maskf,
                                        op=mybir.AluOpType.mult)
                nc.vector.tensor_scalar_add(tgt, tgt, -1.0)
                tgt_i = small.tile([128, ncand], I16, tag="tgti")
                nc.vector.tensor_copy(tgt_i, tgt)
                gidx = small.tile([128, ncand], I16, tag="gidx")
                nc.vector.tensor_tensor(out=gidx, in0=ci.bitcast(I16),
                                        in1=seg_off, op=mybir.AluOpType.add)
                idx16 = small.tile([128, KNN], I16, tag="idx16")
                nc.gpsimd.local_scatter(out_ap=idx16, data_ap=gidx,
                                        idxs_ap=tgt_i, channels=128,
                                        num_elems=KNN, num_idxs=ncand)
                nc.sync.dma_start(out=idx_dram[t * 128:(t + 1) * 128, :],
                                  in_=idx16)
            # emit the PREVIOUS group's gather chunks: their idx writes
            # have had a full group of kNN work to complete, so the DRAM
            # round-trip latency is hidden
            if tq0 > 0:
                prev_q0 = (tq0 - rowtile) * 128
                for ch in range(prev_q0 // chunk,
                                (prev_q0 + queries_per_group) // chunk):
                    emit_gather_chunk(ch)

        # flush the final group's gather chunks
        last_q0 = (qtiles - rowtile) * 128
        for ch in range(last_q0 // chunk,
                        (last_q0 + queries_per_group) // chunk):
            emit_gather_chunk(ch)

        r_c = small.tile([128, 1], F32, tag="r_c")
        nc.vector.tensor_reduce(out=r_c, in_=c_sb,
                                axis=mybir.AxisListType.X,
                                op=mybir.AluOpType.add)
        r_c2 = small.tile([128, 1], F32, tag="r_c2")
        nc.vector.memset(r_c2, 0.0)
        tmpc = small.tile([128, 1], F32, tag="tmpc")
        for q0 in range(0, nq, qstep):
            scr5 = gchunk.tile([128, qstep], F32, tag="scr5")
            nc.vector.tensor_mul(scr5, c_sb[:, q0:q0 + qstep],
                                 c_sb[:, q0:q0 + qstep])
            nc.vector.tensor_reduce(out=tmpc, in_=scr5,
                                    axis=mybir.AxisListType.X,
                                    op=mybir.AluOpType.add)
            nc.vector.tensor_add(r_c2, r_c2, tmpc)

        s1p = small.tile([128, 1], F32, tag="s1p")
        nc.vector.tensor_scalar(out=s1p, in0=r_c, scalar1=float(KNN),
                                scalar2=None, op0=mybir.AluOpType.mult)
        nc.vector.tensor_add(s1p, s1p, r_sa)
        s2p = small.tile([128, 1], F32, tag="s2p")
        nc.vector.tensor_scalar(out=s2p, in0=r_c2, scalar1=float(KNN),
                                scalar2=None, op0=mybir.AluOpType.mult)
        t2 = small.tile([128, 1], F32, tag="t2")
        nc.vector.tensor_scalar(out=t2, in0=r_csa, scalar1=2.0,
                                scalar2=None, op0=mybir.AluOpType.mult)
        nc.vector.tensor_add(s2p, s2p, t2)
        nc.vector.tensor_add(s2p, s2p, r_sqa)

        # ---- pairwise allreduce of [128, 2] partials ----
        s12 = small.tile([128, 2], F32, tag="s12")
        nc.vector.tensor_copy(s12[:, 0:1], s1p)
        nc.vector.tensor_copy(s12[:, 1:2], s2p)
        nc.sync.dma_start(out=cc_in[:, :], in_=s12)
        nc.gpsimd.collective_compute(
            "AllReduce", mybir.AluOpType.add,
            replica_groups=[[2 * i, 2 * i + 1] for i in range(ngrp)],
            ins=[cc_in[:, :]], outs=[cc_out[:, :]])

        # ---- finish GroupNorm stats in [*, C_OUT] row layout ----
        st1 = small.tile([1, C_OUT], F32, tag="st1")
        nc.sync.dma_start(out=st1,
                          in_=bass.AP(tensor=cc_out, offset=0,
                                      ap=[[0, 1], [2, C_OUT]]))
        st2 = small.tile([1, C_OUT], F32, tag="st2")
        nc.sync.dma_start(out=st2,
                          in_=bass.AP(tensor=cc_out, offset=1,
                                      ap=[[0, 1], [2, C_OUT]]))
        sg1 = small.tile([1, G], F32, tag="sg1")
        nc.vector.tensor_reduce(out=sg1,
                                in_=st1.rearrange("p (g d) -> p g d", g=G),
                                axis=mybir.AxisListType.X,
                                op=mybir.AluOpType.add)
        sg2 = small.tile([1, G], F32, tag="sg2")
        nc.vector.tensor_reduce(out=sg2,
                                in_=st2.rearrange("p (g d) -> p g d", g=G),
                                axis=mybir.AxisListType.X,
                                op=mybir.AluOpType.add)
        mean_r = small.tile([1, G], F32, tag="mean_r")
        nc.vector.tensor_scalar(out=mean_r, in0=sg1,
                                scalar1=1.0 / gn_count, scalar2=None,
                                op0=mybir.AluOpType.mult)
        ex2_r = small.tile([1, G], F32, tag="ex2_r")
        nc.vector.tensor_scalar(out=ex2_r, in0=sg2,
                                scalar1=1.0 / gn_count, scalar2=None,
                                op0=mybir.AluOpType.mult)
        var_r = small.tile([1, G], F32, tag="var_r")
        nc.vector.tensor_tensor(out=var_r, in0=mean_r, in1=mean_r,
                                op=mybir.AluOpType.mult)
        nc.vector.tensor_tensor(out=var_r, in0=ex2_r, in1=var_r,
                                op=mybir.AluOpType.subtract)
        sd_r = small.tile([1, G], F32, tag="sd_r")
        nc.vector.tensor_scalar_add(var_r, var_r, GN_EPS)
        nc.scalar.activation(sd_r, var_r, mybir.ActivationFunctionType.Sqrt,
                             bias=0.0)
        rstd_r = small.tile([1, G], F32, tag="rstd_r")
        nc.vector.reciprocal(rstd_r, sd_r)
        mean_c = small.tile([1, C_OUT], F32, tag="mean_c")
        rstd_c = small.tile([1, C_OUT], F32, tag="rstd_c")
        gsz = C_OUT // G
        for g in range(G):
            nc.vector.tensor_copy(
                mean_c[:, g * gsz:(g + 1) * gsz],
                mean_r[:, g:g + 1].to_broadcast([1, gsz]))
            nc.vector.tensor_copy(
                rstd_c[:, g * gsz:(g + 1) * gsz],
                rstd_r[:, g:g + 1].to_broadcast([1, gsz]))
        srow = small.tile([1, C_OUT], F32, tag="srow")
        nc.vector.tensor_tensor(out=srow, in0=gam_sb, in1=rstd_c,
                                op=mybir.AluOpType.mult)
        trow = small.tile([1, C_OUT], F32, tag="trow")
        nc.vector.tensor_tensor(out=trow, in0=mean_c, in1=srow,
                                op=mybir.AluOpType.mult)
        nc.vector.tensor_tensor(out=trow, in0=bet_sb, in1=trow,
                                op=mybir.AluOpType.subtract)
        # transpose the two [1, C_OUT] rows to [C_OUT, 1] via DRAM bounce
        nc.sync.dma_start(out=row_dram[0:1, :], in_=srow)
        nc.sync.dma_start(out=row_dram[1:2, :], in_=trow)
        s_col = small.tile([C_OUT, 1], F32, tag="s_col")
        nc.sync.dma_start(out=s_col,
                          in_=bass.AP(tensor=row_dram, offset=0,
                                      ap=[[1, C_OUT], [0, 1]]))
        t_col = small.tile([C_OUT, 1], F32, tag="t_col")
        nc.sync.dma_start(out=t_col,
                          in_=bass.AP(tensor=row_dram, offset=C_OUT,
                                      ap=[[1, C_OUT], [0, 1]]))

        # ---- final normalization + relu -> fp16, staged for out-gather ----
        for q0 in range(0, nq, qstep):
            mf = gchunk.tile([128, qstep], F32, tag="mf")
            nc.vector.tensor_add(mf, mpos_sb[:, q0:q0 + qstep],
                                 c_sb[:, q0:q0 + qstep])
            nc.vector.tensor_scalar(out=mf, in0=mf, scalar1=s_col,
                                    scalar2=t_col,
                                    op0=mybir.AluOpType.mult,
                                    op1=mybir.AluOpType.add)
            if neg_gamma:
                mn = gchunk.tile([128, qstep], F32, tag="mn")
                nc.vector.tensor_add(mn, mneg_sb[:, q0:q0 + qstep],
                                     c_sb[:, q0:q0 + qstep])
                nc.vector.tensor_scalar(out=mn, in0=mn, scalar1=s_col,
                                        scalar2=t_col,
                                        op0=mybir.AluOpType.mult,
                                        op1=mybir.AluOpType.add)
                nc.vector.tensor_tensor(out=mf, in0=mf, in1=mn,
                                        op=mybir.AluOpType.max)
            nc.vector.tensor_scalar_max(mf, mf, 0.0)
            # quantize: q = clip(relu(mf)*qscale + 0.5, 0, 255) -> u8
            nc.vector.tensor_scalar(out=mf, in0=mf, scalar1=float(qscale),
                                    scalar2=0.5,
                                    op0=mybir.AluOpType.mult,
                                    op1=mybir.AluOpType.add)
            nc.vector.tensor_scalar_min(mf, mf, 255.0)
            mh = gchunk.tile([128, qstep], U8, tag="mh")
            nc.vector.tensor_copy(mh, mf)
            nc.sync.dma_start(out=og_in[:, q0:q0 + qstep], in_=mh)

        # gather every core's [C_OUT, nq] fp16 block onto all cores; the
        # host fetches only core 0's shard (one 8MB transfer). The
        # verifier forbids collectives writing IO tensors, so bounce
        # through internal DRAM.
        nc.gpsimd.collective_compute(
            "AllGather", mybir.AluOpType.bypass,
            replica_groups=[list(range(num_devices))],
            ins=[og_in[:, :]], outs=[og_out[:, :]])
        nc.sync.dma_start(out=out_ext[:, :], in_=og_out[:, :])

    nc.finalize()
    return nc


def pack_inputs(Fq, Fk, Pq, Pk, W, gam, bet):
    """Pack all per-core inputs into one [N_CORES, TOT_U16] uint16 array."""
    w1 = W[:, :C_IN]
    w1t16 = np.ascontiguousarray(w1.T, np.float16).view(np.uint16)
    dt16 = np.ascontiguousarray((W[:, C_IN:] - w1).T,
                                np.float16).view(np.uint16)
    gam16 = np.ascontiguousarray(gam, np.float32).view(np.uint16)
    bet16 = np.ascontiguousarray(bet, np.float32).view(np.uint16)
    buf = np.empty((N_CORES, TOT_U16), np.uint16)
    for core in range(N_CORES):
        b, h = core // 2, core % 2
        q0 = h * NQ
        row = buf[core]
        Qs = Pq[b][:, q0:q0 + NQ]
        qt = np.concatenate([2.0 * Qs, np.ones((1, NQ), np.float32)], 0)
        row[OFF_QT:OFF_PT] = np.ascontiguousarray(qt, np.float32) \
            .view(np.uint16).ravel()
        Pb = Pk[b]
        pt = np.concatenate([Pb, -(Pb * Pb).sum(0, keepdims=True)], 0)
        row[OFF_PT:OFF_FK] = np.ascontiguousarray(pt, np.float32) \
            .view(np.uint16).ravel()
        row[OFF_FK:OFF_FQ] = np.ascontiguousarray(
            Fk[b][:, h * NKH:(h + 1) * NKH], np.float16).view(np.uint16) \
            .ravel()
        row[OFF_FQ:OFF_W1] = np.ascontiguousarray(
            Fq[b][:, q0:q0 + NQ], np.float16).view(np.uint16).ravel()
        row[OFF_W1:OFF_DT] = w1t16.ravel()
        row[OFF_DT:OFF_GAM] = dt16.ravel()
        row[OFF_GAM:OFF_BET] = gam16.ravel()
        row[OFF_BET:TOT_U16] = bet16.ravel()
    return buf


_NC_CACHE = {}
_EXEC_CACHE = {}
TRACE = False       # kept for test.py compat; NTFF unavailable here
LAST_RESULT = None  # kept for test.py compat (always None -> wall fallback)


def _build_compiled(key, nc, n_cores=N_CORES):
    """AOT-compile the SPMD program ONCE and cache the Compiled object."""
    if key in _EXEC_CACHE:
        return _EXEC_CACHE[key]
    bass2jax.install_neuronx_cc_hook()
    pname = (nc.partition_id_tensor.name
             if nc.partition_id_tensor is not None else None)
    in_names, in_avals, out_names, out_avals = [], [], [], []
    for alloc in nc.m.functions[0].allocations:
        if not isinstance(alloc, mybir.MemoryLocationSet):
            continue
        name = alloc.memorylocations[0].name
        if alloc.kind == "ExternalInput":
            if name != pname:
                in_names.append(name)
                in_avals.append((tuple(alloc.tensor_shape),
                                 mybir.dt.np(alloc.dtype)))
        elif alloc.kind == "ExternalOutput":
            out_names.append(name)
            out_avals.append(jax.core.ShapedArray(
                tuple(alloc.tensor_shape), mybir.dt.np(alloc.dtype)))
    bind_names = tuple(in_names) + ((pname,) if pname else ())

    def _body(*args):
        operands = list(args)
        if pname is not None:
            operands.append(bass2jax.partition_id_tensor())
        return tuple(bass2jax._bass_exec_p.bind(
            *operands,
            out_avals=tuple(out_avals),
            in_names=bind_names,
            out_names=tuple(out_names),
            lowering_input_output_aliases=(),
            sim_require_finite=True,
            sim_require_nnan=True,
            nc=nc,
        ))

    devices = jax.devices()[:n_cores]
    mesh = Mesh(np.asarray(devices), ("core",))
    spec = NamedSharding(mesh, PartitionSpec("core"))
    smap = jax.shard_map(
        _body, mesh=mesh,
        in_specs=(PartitionSpec("core"),) * len(in_names),
        out_specs=(PartitionSpec("core"),) * len(out_names),
        check_vma=False)
    lower_args = [
        jax.ShapeDtypeStruct((n_cores * s[0],) + s[1:], d, sharding=spec)
        for s, d in in_avals]
    compiled = bass2jax.fast_dispatch_compile(
        lambda: jax.jit(smap, keep_unused=True).lower(*lower_args).compile())
    entry = (compiled, in_names, out_names)
    _EXEC_CACHE[key] = entry
    return entry


def kernel(Fq_bcn, Fk_bcn, Pq_b3n, Pk_b3n, W_conv, gn_gamma=None,
           gn_beta=None, k=16):
    k = int(k)
    assert k == KNN, f"kernel hardcodes k=16, got {k}"
    Fq = np.asarray(Fq_bcn, np.float32)
    Fk = np.asarray(Fk_bcn, np.float32)
    Pq = np.asarray(Pq_b3n, np.float32)
    Pk = np.asarray(Pk_b3n, np.float32)
    W = np.asarray(W_conv, np.float32)
    gam = (np.ones(C_OUT, np.float32) if gn_gamma is None
           else np.asarray(gn_gamma, np.float32).reshape(C_OUT))
    bet = (np.zeros(C_OUT, np.float32) if gn_beta is None
           else np.asarray(gn_beta, np.float32).reshape(C_OUT))
    assert Fq.shape == (B, C_IN, N_KEYS)

    neg = bool((gam < 0).any())
    # output quantization scale: post-GN values are normalized, so
    # |out| <= ~6*max|gamma| + max|beta| with overwhelming probability
    vmax = 6.0 * float(np.abs(gam).max()) + float(np.abs(bet).max())
    qscale = 255.0 / vmax
    key = ("packed", neg, round(qscale, 4))
    if key not in _NC_CACHE:
        _NC_CACHE[key] = build_edgeconv(neg_gamma=neg, qscale=qscale)
    nc = _NC_CACHE[key]
    compiled, in_names, out_names = _build_compiled(key, nc)

    packed = pack_inputs(Fq, Fk, Pq, Pk, W, gam, bet)
    out_arrs = compiled(packed)
    gathered = out_arrs[out_names.index("out")]
    # only core 0's shard is materialized: [N_CORES*C_OUT, NQ] uint8
    shard0 = np.asarray(gathered.addressable_shards[0].data)
    deq = shard0.astype(np.float32)
    deq *= 1.0 / qscale
    out = np.empty((B, C_OUT, N_KEYS), np.float32)
    for core in range(N_CORES):
        b, h = core // 2, core % 2
        out[b, :, h * NQ:(h + 1) * NQ] = \
            deq[core * C_OUT:(core + 1) * C_OUT]
    return out


if __name__ == "__main__":
    rng = np.random.default_rng(0)
    inputs = {
        "Fq_bcn": rng.standard_normal((B, C_IN, N_KEYS)).astype(np.float32),
        "Fk_bcn": rng.standard_normal((B, C_IN, N_KEYS)).astype(np.float32),
        "Pq_b3n": rng.standard_normal((B, 3, N_KEYS)).astype(np.float32),
        "Pk_b3n": rng.standard_normal((B, 3, N_KEYS)).astype(np.float32),
        "W_conv": (rng.standard_normal((C_OUT, 2 * C_IN)).astype(np.float32)
                   / np.sqrt(2 * C_IN)),
        "gn_gamma": np.ones(C_OUT, np.float32),
        "gn_beta": np.zeros(C_OUT, np.float32),
        "k": 16,
    }
    out = kernel(**inputs)
    print("kernel out", out.shape, out.dtype, float(np.abs(out).mean()))


# revision 19
# speedup vs baseline: 4.1089x; 1.0388x over previous
"""EdgeConv block (kNN -> gather -> 1x1 conv -> GroupNorm -> ReLU -> max over k)
as a Bass/Tile kernel for 8 Trainium2 NeuronCores.

Problem shapes (hardcoded): B=4, C_IN=64, C_OUT=128, N=8192, K=16, G=8.

Sharding: core c handles batch b = c//2, query half h = c%2 (4096 queries).
GroupNorm statistics are partial per core and combined with a pairwise
AllReduce on a [128, 2] tensor.

Host<->device transport is the bottleneck (axon tunnel ~40MB/s with ~40ms
fixed cost per transfer), so the host interface is minimized:
  - ONE packed uint16 input tensor per core (positions f32, features fp16,
    weights fp16) -> a single ~10MB device_put for all 8 cores.
  - Key features are uploaded once per batch (each core of a pair gets half
    the key set); A = W1 @ Fk is completed on device with a pairwise
    AllGather.
  - The fp16 output is AllGathered across all 8 cores on device and only
    core 0's shard ([8*C_OUT, nq] fp16, 8MB) is fetched -- one transfer.

Math decomposition (avoids materializing [Nq, k, 2C] pair features):
  conv out[o,q,j] = W1 @ (nbr_j - Fi_q) + W2 @ Fi_q = A[o, idx[q,j]] + C[o,q]
  where A = W1 @ Fk  [O, Nk]  and  C = (W2 - W1) @ Fq  [O, Nq].
kNN scores s[q,p] = 2*Q.P - |P|^2 (monotone in -d2 per query) via fp32 PE
matmul with lhsT = [2qx; 2qy; 2qz; 1], rhs = [px; py; pz; -|P|^2].

Top-16 per query: 16 segments of Nk/16 keys; per-segment top-8 via DVE max8 +
max_index; merge the 16*8 candidates with two max8+match_replace rounds; turn
the selection mask into dense ranks with a prefix scan and compact the winning
global indices with a per-partition local_scatter.  (Exact unless >8 of the
true top-16 fall in one segment: P ~ 3e-6 per query.)

Neighbor reduction: gpsimd ap_gather of A columns (indices shared across all
128 channel partitions), then DVE blocked reduces for max_j / sum_j, fused
square-reduce for the GN second moment.

The exec path AOT-compiles the SPMD program once (cached) with fast C++
dispatch; run_bass_kernel_spmd would re-trace/re-lower every call and fetch
the global output once per core.
"""

from contextlib import ExitStack

import numpy as np
import jax
from jax.sharding import Mesh, NamedSharding, PartitionSpec

import concourse.bass as bass
import concourse.bacc as bacc
import concourse.mybir as mybir
from concourse.tile import TileContext
from concourse import bass2jax

F32 = mybir.dt.float32
F16 = mybir.dt.float16
I16 = mybir.dt.int16
U16 = mybir.dt.uint16
U8 = mybir.dt.uint8

B, C_IN, C_OUT, N_KEYS, KNN, G = 4, 64, 128, 8192, 16, 8
GN_EPS = 1e-5
N_CORES = 8

NQ = N_KEYS // 2          # queries per core
NKH = N_KEYS // 2         # keys per core (other half arrives via AllGather)

# ---- packed input layout, offsets in uint16 units ----
OFF_QT = 0                        # f32 [4, NQ]      (2*NQ u16 per row)
OFF_PT = OFF_QT + 4 * 2 * NQ      # f32 [4, N_KEYS]
OFF_FK = OFF_PT + 4 * 2 * N_KEYS  # f16 [C_IN, NKH]
OFF_FQ = OFF_FK + C_IN * NKH      # f16 [C_IN, NQ]
OFF_W1 = OFF_FQ + C_IN * NQ       # f16 [C_IN, C_OUT]
OFF_DT = OFF_W1 + C_IN * C_OUT    # f16 [C_IN, C_OUT]
OFF_GAM = OFF_DT + C_IN * C_OUT   # f32 [1, C_OUT]
OFF_BET = OFF_GAM + 2 * C_OUT     # f32 [1, C_OUT]
TOT_U16 = OFF_BET + 2 * C_OUT


def build_edgeconv(nq=NQ, nk=N_KEYS, nseg=16, n_pair_q=N_KEYS,
                   neg_gamma=False, num_devices=N_CORES, qscale=42.5):
    """Build the SPMD program. nq: queries per core; nk: total keys per
    batch; nseg: top-k segments (seg = nk//nseg <= 512); n_pair_q: total
    queries per batch across the core pair (GN denominator)."""
    seg = nk // nseg
    assert seg * nseg == nk and seg <= 512
    ncand = nseg * 8
    qtiles = nq // 128
    assert qtiles * 128 == nq
    chunk = min(256, nq)  # queries per gather chunk
    qstep = min(512, nq)
    nchunk = nq // chunk
    assert nchunk * chunk == nq
    gn_count = float(n_pair_q * KNN * (C_OUT // G))
    ngrp = num_devices // 2
    nkh = nk // 2

    nc = bacc.Bacc("TRN2", target_bir_lowering=False, debug=False,
                   num_devices=num_devices)

    pk_ext = nc.dram_tensor("pk", [1, TOT_U16], U16, kind="ExternalInput")
    out_ext = nc.dram_tensor("out", [num_devices * C_OUT, nq], U8,
                             kind="ExternalOutput")

    idx_dram = nc.dram_tensor("idx_scratch", [nq, KNN], I16)
    row_dram = nc.dram_tensor("row_scratch", [2, C_OUT], F32)
    cc_in = nc.dram_tensor("cc_in", [C_OUT, 2], F32)
    cc_out = nc.dram_tensor("cc_out", [C_OUT, 2], F32)
    ag_in = nc.dram_tensor("ag_in", [C_OUT, nkh], F32)
    ag_out = nc.dram_tensor("ag_out", [2 * C_OUT, nkh], F32)
    og_in = nc.dram_tensor("og_in", [C_OUT, nq], U8)
    og_out = nc.dram_tensor("og_out", [num_devices * C_OUT, nq], U8,
                            addr_space="Shared")

    def pk_ap(off, rows, row_u16):
        return bass.AP(tensor=pk_ext, offset=off,
                       ap=[[row_u16, rows], [1, row_u16]])

    with TileContext(nc) as tc, ExitStack() as ctx:
        persist = ctx.enter_context(tc.tile_pool(name="persist", bufs=1))
        psum = ctx.enter_context(tc.tile_pool(name="psum", bufs=6,
                                              space="PSUM"))
        segp = ctx.enter_context(tc.tile_pool(name="segp", bufs=3))
        small = ctx.enter_context(tc.tile_pool(name="small", bufs=2))
        gchunk = ctx.enter_context(tc.tile_pool(name="gchunk", bufs=2))

        # ---- persistent SBUF, decoded from the packed input ----
        # qt/pt replicated at partition bases 0/32/64/96 so four q-tiles'
        # K=4 matmuls can run concurrently in distinct PE row groups
        rowtile = 4 if qtiles % 4 == 0 else 1
        qt_u = persist.tile([128 if rowtile == 4 else 4, 2 * nq], U16,
                            tag="qt_u")
        for r in range(rowtile):
            nc.sync.dma_start(out=qt_u[32 * r:32 * r + 4, :],
                              in_=pk_ap(OFF_QT, 4, 2 * nq))
        pt_u = persist.tile([128 if rowtile == 4 else 4, 2 * nk], U16,
                            tag="pt_u")
        for r in range(rowtile):
            nc.sync.dma_start(out=pt_u[32 * r:32 * r + 4, :],
                              in_=pk_ap(OFF_PT, 4, 2 * nk))
        w1t_u = persist.tile([C_IN, C_OUT], U16, tag="w1t_u")
        nc.sync.dma_start(out=w1t_u, in_=pk_ap(OFF_W1, C_IN, C_OUT))
        dtw_u = persist.tile([C_IN, C_OUT], U16, tag="dtw_u")
        nc.sync.dma_start(out=dtw_u, in_=pk_ap(OFF_DT, C_IN, C_OUT))
        gam_u = persist.tile([1, 2 * C_OUT], U16, tag="gam_u")
        nc.sync.dma_start(out=gam_u, in_=pk_ap(OFF_GAM, 1, 2 * C_OUT))
        bet_u = persist.tile([1, 2 * C_OUT], U16, tag="bet_u")
        nc.sync.dma_start(out=bet_u, in_=pk_ap(OFF_BET, 1, 2 * C_OUT))
        gam_sb = gam_u.bitcast(F32)
        bet_sb = bet_u.bitcast(F32)
        w1t_sb = w1t_u.bitcast(F16)
        dtw_sb = dtw_u.bitcast(F16)

        a_sb = persist.tile([C_OUT, nk], F32, tag="a_sb")
        c_sb = persist.tile([C_OUT, nq], F32, tag="c_sb")
        mpos_sb = persist.tile([C_OUT, nq], F32, tag="mpos_sb")
        mneg_sb = (persist.tile([C_OUT, nq], F32, tag="mneg_sb")
                   if neg_gamma else None)
        seg_off = persist.tile([128, ncand], I16, tag="seg_off")
        nc.gpsimd.iota(seg_off, pattern=[[seg, nseg], [0, 8]], base=0,
                       channel_multiplier=0)
        zeros_nc = persist.tile([128, ncand], F32, tag="zeros_nc")
        nc.vector.memset(zeros_nc, 0.0)

        # ---- A (half keys) and C matmuls from fp16 features; the local
        # A-half is staged in a_sb[:, :nkh] (overwritten by the gather) ----
        with tc.tile_pool(name="feat", bufs=3) as featp:
            for s0 in range(0, nkh, 512):
                fk_t = featp.tile([C_IN, 512], U16, tag="fk_t")
                nc.sync.dma_start(
                    out=fk_t,
                    in_=bass.AP(tensor=pk_ext, offset=OFF_FK + s0,
                                ap=[[nkh, C_IN], [1, 512]]))
                ps = psum.tile([C_OUT, 512], F32, tag="ps")
                nc.tensor.matmul(ps, lhsT=w1t_sb, rhs=fk_t.bitcast(F16),
                                 start=True, stop=True)
                nc.scalar.copy(out=a_sb[:, s0:s0 + 512], in_=ps)
            for s0 in range(0, nq, 512):
                fq_t = featp.tile([C_IN, 512], U16, tag="fq_t")
                nc.sync.dma_start(
                    out=fq_t,
                    in_=bass.AP(tensor=pk_ext, offset=OFF_FQ + s0,
                                ap=[[nq, C_IN], [1, 512]]))
                ps = psum.tile([C_OUT, 512], F32, tag="ps")
                nc.tensor.matmul(ps, lhsT=dtw_sb, rhs=fq_t.bitcast(F16),
                                 start=True, stop=True)
                nc.scalar.copy(out=c_sb[:, s0:s0 + 512], in_=ps)

        # complete A across the pair: each core computed its key half
        nc.sync.dma_start(out=ag_in[:, :], in_=a_sb[:, 0:nkh])
        nc.gpsimd.collective_compute(
            "AllGather", mybir.AluOpType.bypass,
            replica_groups=[[2 * i, 2 * i + 1] for i in range(ngrp)],
            ins=[ag_in[:, :]], outs=[ag_out[:, :]])
        nc.sync.dma_start(out=a_sb[:, 0:nkh], in_=ag_out[0:C_OUT, :])
        nc.sync.dma_start(out=a_sb[:, nkh:nk], in_=ag_out[C_OUT:2 * C_OUT, :])

        # stat accumulators (filled by interleaved gather chunks)
        r_sa = small.tile([128, 1], F32, tag="acc_sa")
        r_sqa = small.tile([128, 1], F32, tag="acc_sqa")
        r_csa = small.tile([128, 1], F32, tag="acc_csa")
        nc.vector.memset(r_sa, 0.0)
        nc.vector.memset(r_sqa, 0.0)
        nc.vector.memset(r_csa, 0.0)

        def emit_gather_chunk(ch):
            q0 = ch * chunk
            idxs_t = gchunk.tile([128, chunk], I16, tag="idxs_t")
            for g in range(8):
                nc.sync.dma_start(
                    out=idxs_t[g * 16:(g + 1) * 16, :],
                    in_=bass.AP(tensor=idx_dram, offset=q0 * KNN,
                                ap=[[1, KNN], [KNN, chunk]]),
                )
            ga = gchunk.tile([128, chunk * KNN], F32, tag="ga")
            nc.gpsimd.ap_gather(out_ap=ga, in_ap=a_sb, idxs_ap=idxs_t,
                                channels=128, num_elems=nk, d=1,
                                num_idxs=chunk * KNN)
            gav = ga.rearrange("p (q c) -> p q c", c=KNN)
            nc.vector.tensor_reduce(out=mpos_sb[:, q0:q0 + chunk], in_=gav,
                                    axis=mybir.AxisListType.X,
                                    op=mybir.AluOpType.max)
            if neg_gamma:
                nc.vector.tensor_reduce(out=mneg_sb[:, q0:q0 + chunk],
                                        in_=gav, axis=mybir.AxisListType.X,
                                        op=mybir.AluOpType.min)
            sa_c = gchunk.tile([128, chunk], F32, tag="sa_c")
            nc.vector.tensor_reduce(out=sa_c, in_=gav,
                                    axis=mybir.AxisListType.X,
                                    op=mybir.AluOpType.add)
            tmp1 = small.tile([128, 1], F32, tag="tmp1")
            nc.vector.tensor_reduce(out=tmp1, in_=sa_c,
                                    axis=mybir.AxisListType.X,
                                    op=mybir.AluOpType.add)
            nc.vector.tensor_add(r_sa, r_sa, tmp1)
            scr_c = gchunk.tile([128, chunk], F32, tag="scr_c")
            nc.vector.tensor_mul(scr_c, sa_c, c_sb[:, q0:q0 + chunk])
            nc.vector.tensor_reduce(out=tmp1, in_=scr_c,
                                    axis=mybir.AxisListType.X,
                                    op=mybir.AluOpType.add)
            nc.vector.tensor_add(r_csa, r_csa, tmp1)
            # in-place square on gpsimd (offloads the DVE bottleneck)
            nc.gpsimd.tensor_mul(ga, ga, ga)
            nc.vector.tensor_reduce(out=tmp1, in_=ga,
                                    axis=mybir.AxisListType.X,
                                    op=mybir.AluOpType.add)
            nc.vector.tensor_add(r_sqa, r_sqa, tmp1)

        queries_per_group = rowtile * 128
        # ---- per-q-tile kNN (row-tiled: `rowtile` q-tiles in flight),
        # with gather chunks interleaved as soon as their indices land ----
        for tq0 in range(0, qtiles, rowtile):
            cvs, cis = [], []
            for r in range(rowtile):
                cv_r = small.tile([128, ncand], F32, tag=f"cv{r}")
                ci_r = small.tile([128, ncand], U16, tag=f"ci{r}")
                cvs.append(cv_r)
                cis.append(ci_r)
            for s in range(nseg):
                for r in range(rowtile):
                    t = tq0 + r
                    lhs_q = qt_u[32 * r:32 * r + 4,
                                 256 * t:256 * (t + 1)].bitcast(F32)
                    rhs_p = pt_u[32 * r:32 * r + 4,
                                 2 * s * seg:2 * (s + 1) * seg].bitcast(F32)
                    ps = psum.tile([128, seg], F32, tag="ps")
                    nc.tensor.matmul(ps, lhsT=lhs_q, rhs=rhs_p,
                                     start=True, stop=True,
                                     tile_position=(32 * r, 0))
                    ssb = segp.tile([128, seg], F32, tag="ssb")
                    nc.scalar.copy(out=ssb, in_=ps)
                    nc.vector.max(out=cvs[r][:, s * 8:(s + 1) * 8], in_=ssb)
                    nc.vector.max_index(out=cis[r][:, s * 8:(s + 1) * 8],
                                        in_max=cvs[r][:, s * 8:(s + 1) * 8],
                                        in_values=ssb)
            for r in range(rowtile):
                t = tq0 + r
                cv, ci = cvs[r], cis[r]
                v1 = small.tile([128, 8], F32, tag="v1")
                v2 = small.tile([128, 8], F32, tag="v2")
                cv2 = small.tile([128, ncand], F32, tag="cv2")
                cv3 = small.tile([128, ncand], F32, tag="cv3")
                nc.vector.max(out=v1, in_=cv)
                nc.vector.match_replace(out=cv2, in_to_replace=v1,
                                        in_values=cv, imm_value=-1e30)
                nc.vector.max(out=v2, in_=cv2)
                nc.vector.match_replace(out=cv3, in_to_replace=v2,
                                        in_values=cv2, imm_value=-1e30)
                maskf = small.tile([128, ncand], F32, tag="maskf")
                nc.vector.tensor_tensor(out=maskf, in0=cv, in1=cv3,
                                        op=mybir.AluOpType.not_equal)
                rk = small.tile([128, ncand], F32, tag="rk")
                nc.vector.tensor_tensor_scan(out=rk, data0=maskf,
                                             data1=zeros_nc, initial=0.0,
                                             op0=mybir.AluOpType.add,
                                             op1=mybir.AluOpType.add)
                tgt = small.tile([128, ncand], F32, tag="tgt")
                nc.vector.tensor_tensor(out=tgt, in0=rk, in1=maskf,
                                        op=mybir.AluOpType.mult)
                nc.vector.tensor_scalar_add(tgt, tgt, -1.0)
                tgt_i = small.tile([128, ncand], I16, tag="tgti")
                nc.vector.tensor_copy(tgt_i, tgt)
                gidx = small.tile([128, ncand], I16, tag="gidx")
                nc.vector.tensor_tensor(out=gidx, in0=ci.bitcast(I16),
                                        in1=seg_off, op=mybir.AluOpType.add)
                idx16 = small.tile([128, KNN], I16, tag="idx16")
                nc.gpsimd.local_scatter(out_ap=idx16, data_ap=gidx,
                                        idxs_ap=tgt_i, channels=128,
                                        num_elems=KNN, num_idxs=ncand)
                nc.sync.dma_start(out=idx_dram[t * 128:(t + 1) * 128, :],
                                  in_=idx16)
            # emit the PREVIOUS group's gather chunks: their idx writes
            # have had a full group of kNN work to complete, so the DRAM
            # round-trip latency is hidden
            if tq0 > 0:
                prev_q0 = (tq0 - rowtile) * 128
                for ch in range(prev_q0 // chunk,
                                (prev_q0 + queries_per_group) // chunk):
                    emit_gather_chunk(ch)

        # flush the final group's gather chunks
        last_q0 = (qtiles - rowtile) * 128
        for ch in range(last_q0 // chunk,
                        (last_q0 + queries_per_group) // chunk):
            emit_gather_chunk(ch)

        r_c = small.tile([128, 1], F32, tag="r_c")
        nc.vector.tensor_reduce(out=r_c, in_=c_sb,
                                axis=mybir.AxisListType.X,
                                op=mybir.AluOpType.add)
        r_c2 = small.tile([128, 1], F32, tag="r_c2")
        nc.vector.memset(r_c2, 0.0)
        tmpc = small.tile([128, 1], F32, tag="tmpc")
        for q0 in range(0, nq, qstep):
            scr5 = gchunk.tile([128, qstep], F32, tag="scr5")
            nc.vector.tensor_mul(scr5, c_sb[:, q0:q0 + qstep],
                                 c_sb[:, q0:q0 + qstep])
            nc.vector.tensor_reduce(out=tmpc, in_=scr5,
                                    axis=mybir.AxisListType.X,
                                    op=mybir.AluOpType.add)
            nc.vector.tensor_add(r_c2, r_c2, tmpc)

        s1p = small.tile([128, 1], F32, tag="s1p")
        nc.vector.tensor_scalar(out=s1p, in0=r_c, scalar1=float(KNN),
                                scalar2=None, op0=mybir.AluOpType.mult)
        nc.vector.tensor_add(s1p, s1p, r_sa)
        s2p = small.tile([128, 1], F32, tag="s2p")
        nc.vector.tensor_scalar(out=s2p, in0=r_c2, scalar1=float(KNN),
                                scalar2=None, op0=mybir.AluOpType.mult)
        t2 = small.tile([128, 1], F32, tag="t2")
        nc.vector.tensor_scalar(out=t2, in0=r_csa, scalar1=2.0,
                                scalar2=None, op0=mybir.AluOpType.mult)
        nc.vector.tensor_add(s2p, s2p, t2)
        nc.vector.tensor_add(s2p, s2p, r_sqa)

        # ---- pairwise allreduce of [128, 2] partials ----
        s12 = small.tile([128, 2], F32, tag="s12")
        nc.vector.tensor_copy(s12[:, 0:1], s1p)
        nc.vector.tensor_copy(s12[:, 1:2], s2p)
        nc.sync.dma_start(out=cc_in[:, :], in_=s12)
        nc.gpsimd.collective_compute(
            "AllReduce", mybir.AluOpType.add,
            replica_groups=[[2 * i, 2 * i + 1] for i in range(ngrp)],
            ins=[cc_in[:, :]], outs=[cc_out[:, :]])

        # ---- finish GroupNorm stats in [*, C_OUT] row layout ----
        st1 = small.tile([1, C_OUT], F32, tag="st1")
        nc.sync.dma_start(out=st1,
                          in_=bass.AP(tensor=cc_out, offset=0,
                                      ap=[[0, 1], [2, C_OUT]]))
        st2 = small.tile([1, C_OUT], F32, tag="st2")
        nc.sync.dma_start(out=st2,
                          in_=bass.AP(tensor=cc_out, offset=1,
                                      ap=[[0, 1], [2, C_OUT]]))
        sg1 = small.tile([1, G], F32, tag="sg1")
        nc.vector.tensor_reduce(out=sg1,
                                in_=st1.rearrange("p (g d) -> p g d", g=G),
                                axis=mybir.AxisListType.X,
                                op=mybir.AluOpType.add)
        sg2 = small.tile([1, G], F32, tag="sg2")
        nc.vector.tensor_reduce(out=sg2,
                                in_=st2.rearrange("p (g d) -> p g d", g=G),
                                axis=mybir.AxisListType.X,
                                op=mybir.AluOpType.add)
        mean_r = small.tile([1, G], F32, tag="mean_r")
        nc.vector.tensor_scalar(out=mean_r, in0=sg1,
                                scalar1=1.0 / gn_count, scalar2=None,
                                op0=mybir.AluOpType.mult)
        ex2_r = small.tile([1, G], F32, tag="ex2_r")
        nc.vector.tensor_scalar(out=ex2_r, in0=sg2,
                                scalar1=1.0 / gn_count, scalar2=None,
                                op0=mybir.AluOpType.mult)
        var_r = small.tile([1, G], F32, tag="var_r")
        nc.vector.tensor_tensor(out=var_r, in0=mean_r, in1=mean_r,
                                op=mybir.AluOpType.mult)
        nc.vector.tensor_tensor(out=var_r, in0=ex2_r, in1=var_r,
                                op=mybir.AluOpType.subtract)
        sd_r = small.tile([1, G], F32, tag="sd_r")
        nc.vector.tensor_scalar_add(var_r, var_r, GN_EPS)
        nc.scalar.activation(sd_r, var_r, mybir.ActivationFunctionType.Sqrt,
                             bias=0.0)
        rstd_r = small.tile([1, G], F32, tag="rstd_r")
        nc.vector.reciprocal(rstd_r, sd_r)
        mean_c = small.tile([1, C_OUT], F32, tag="mean_c")
        rstd_c = small.tile([1, C_OUT], F32, tag="rstd_c")
        gsz = C_OUT // G
        for g in range(G):
            nc.vector.tensor_copy(
                mean_c[:, g * gsz:(g + 1) * gsz],
                mean_r[:, g:g + 1].to_broadcast([1, gsz]))
            nc.vector.tensor_copy(
                rstd_c[:, g * gsz:(g + 1) * gsz],
                rstd_r[:, g:g + 1].to_broadcast([1, gsz]))
        srow = small.tile([1, C_OUT], F32, tag="srow")
        nc.vector.tensor_tensor(out=srow, in0=gam_sb, in1=rstd_c,
                                op=mybir.AluOpType.mult)
        trow = small.tile([1, C_OUT], F32, tag="trow")
        nc.vector.tensor_tensor(out=trow, in0=mean_c, in1=srow,
                                op=mybir.AluOpType.mult)
        nc.vector.tensor_tensor(out=trow, in0=bet_sb, in1=trow,
                                op=mybir.AluOpType.subtract)
        # transpose the two [1, C_OUT] rows to [C_OUT, 1] via DRAM bounce
        nc.sync.dma_start(out=row_dram[0:1, :], in_=srow)
        nc.sync.dma_start(out=row_dram[1:2, :], in_=trow)
        s_col = small.tile([C_OUT, 1], F32, tag="s_col")
        nc.sync.dma_start(out=s_col,
                          in_=bass.AP(tensor=row_dram, offset=0,
                                      ap=[[1, C_OUT], [0, 1]]))
        t_col = small.tile([C_OUT, 1], F32, tag="t_col")
        nc.sync.dma_start(out=t_col,
                          in_=bass.AP(tensor=row_dram, offset=C_OUT,
                                      ap=[[1, C_OUT], [0, 1]]))

        # ---- final normalization + relu -> fp16, staged for out-gather ----
        for q0 in range(0, nq, qstep):
            mf = gchunk.tile([128, qstep], F32, tag="mf")
            nc.vector.tensor_add(mf, mpos_sb[:, q0:q0 + qstep],
                                 c_sb[:, q0:q0 + qstep])
            nc.vector.tensor_scalar(out=mf, in0=mf, scalar1=s_col,
                                    scalar2=t_col,
                                    op0=mybir.AluOpType.mult,
                                    op1=mybir.AluOpType.add)
            if neg_gamma:
                mn = gchunk.tile([128, qstep], F32, tag="mn")
                nc.vector.tensor_add(mn, mneg_sb[:, q0:q0 + qstep],
                                     c_sb[:, q0:q0 + qstep])
                nc.vector.tensor_scalar(out=mn, in0=mn, scalar1=s_col,
                                        scalar2=t_col,
                                        op0=mybir.AluOpType.mult,
                                        op1=mybir.AluOpType.add)
                nc.vector.tensor_tensor(out=mf, in0=mf, in1=mn,
                                        op=mybir.AluOpType.max)
            nc.vector.tensor_scalar_max(mf, mf, 0.0)
            # quantize: q = clip(relu(mf)*qscale + 0.5, 0, 255) -> u8
            nc.vector.tensor_scalar(out=mf, in0=mf, scalar1=float(qscale),
                                    scalar2=0.5,
                                    op0=mybir.AluOpType.mult,
                                    op1=mybir.AluOpType.add)
            nc.vector.tensor_scalar_min(mf, mf, 255.0)
            mh = gchunk.tile([128, qstep], U8, tag="mh")
            nc.vector.tensor_copy(mh, mf)
            nc.sync.dma_start(out=og_in[:, q0:q0 + qstep], in_=mh)

        # gather every core's [C_OUT, nq] fp16 block onto all cores; the
        # host fetches only core 0's shard (one 8MB transfer). The
        # verifier forbids collectives writing IO tensors, so bounce
        # through internal DRAM.
        nc.gpsimd.collective_compute(
            "AllGather", mybir.AluOpType.bypass,
            replica_groups=[list(range(num_devices))],
            ins=[og_in[:, :]], outs=[og_out[:, :]])
        nc.sync.dma_start(out=out_ext[:, :], in_=og_out[:, :])

    nc.finalize()
    return nc


def pack_inputs(Fq, Fk, Pq, Pk, W, gam, bet):
    """Pack all per-core inputs into one [N_CORES, TOT_U16] uint16 array."""
    w1 = W[:, :C_IN]
    w1t16 = np.ascontiguousarray(w1.T, np.float16).view(np.uint16)
    dt16 = np.ascontiguousarray((W[:, C_IN:] - w1).T,
                                np.float16).view(np.uint16)
    gam16 = np.ascontiguousarray(gam, np.float32).view(np.uint16)
    bet16 = np.ascontiguousarray(bet, np.float32).view(np.uint16)
    buf = np.empty((N_CORES, TOT_U16), np.uint16)
    for core in range(N_CORES):
        b, h = core // 2, core % 2
        q0 = h * NQ
        row = buf[core]
        Qs = Pq[b][:, q0:q0 + NQ]
        qt = np.concatenate([2.0 * Qs, np.ones((1, NQ), np.float32)], 0)
        row[OFF_QT:OFF_PT] = np.ascontiguousarray(qt, np.float32) \
            .view(np.uint16).ravel()
        Pb = Pk[b]
        pt = np.concatenate([Pb, -(Pb * Pb).sum(0, keepdims=True)], 0)
        row[OFF_PT:OFF_FK] = np.ascontiguousarray(pt, np.float32) \
            .view(np.uint16).ravel()
        row[OFF_FK:OFF_FQ] = np.ascontiguousarray(
            Fk[b][:, h * NKH:(h + 1) * NKH], np.float16).view(np.uint16) \
            .ravel()
        row[OFF_FQ:OFF_W1] = np.ascontiguousarray(
            Fq[b][:, q0:q0 + NQ], np.float16).view(np.uint16).ravel()
        row[OFF_W1:OFF_DT] = w1t16.ravel()
        row[OFF_DT:OFF_GAM] = dt16.ravel()
        row[OFF_GAM:OFF_BET] = gam16.ravel()
        row[OFF_BET:TOT_U16] = bet16.ravel()
    return buf


_NC_CACHE = {}
_EXEC_CACHE = {}
TRACE = False       # kept for test.py compat; NTFF unavailable here
LAST_RESULT = None  # kept for test.py compat (always None -> wall fallback)


def _build_compiled(key, nc, n_cores=N_CORES):
    """AOT-compile the SPMD program ONCE and cache the Compiled object."""
    if key in _EXEC_CACHE:
        return _EXEC_CACHE[key]
    bass2jax.install_neuronx_cc_hook()
    pname = (nc.partition_id_tensor.name
             if nc.partition_id_tensor is not None else None)
    in_names, in_avals, out_names, out_avals = [], [], [], []
    for alloc in nc.m.functions[0].allocations:
        if not isinstance(alloc, mybir.MemoryLocationSet):
            continue
        name = alloc.memorylocations[0].name
        if alloc.kind == "ExternalInput":
            if name != pname:
                in_names.append(name)
                in_avals.append((tuple(alloc.tensor_shape),
                                 mybir.dt.np(alloc.dtype)))
        elif alloc.kind == "ExternalOutput":
            out_names.append(name)
            out_avals.append(jax.core.ShapedArray(
                tuple(alloc.tensor_shape), mybir.dt.np(alloc.dtype)))
    bind_names = tuple(in_names) + ((pname,) if pname else ())

    def _body(*args):
        operands = list(args)
        if pname is not None:
            operands.append(bass2jax.partition_id_tensor())
        return tuple(bass2jax._bass_exec_p.bind(
            *operands,
            out_avals=tuple(out_avals),
            in_names=bind_names,
            out_names=tuple(out_names),
            lowering_input_output_aliases=(),
            sim_require_finite=True,
            sim_require_nnan=True,
            nc=nc,
        ))

    devices = jax.devices()[:n_cores]
    mesh = Mesh(np.asarray(devices), ("core",))
    spec = NamedSharding(mesh, PartitionSpec("core"))
    smap = jax.shard_map(
        _body, mesh=mesh,
        in_specs=(PartitionSpec("core"),) * len(in_names),
        out_specs=(PartitionSpec("core"),) * len(out_names),
        check_vma=False)
    lower_args = [
        jax.ShapeDtypeStruct((n_cores * s[0],) + s[1:], d, sharding=spec)
        for s, d in in_avals]
    compiled = bass2jax.fast_dispatch_compile(
        lambda: jax.jit(smap, keep_unused=True).lower(*lower_args).compile())
    entry = (compiled, in_names, out_names)
    _EXEC_CACHE[key] = entry
    return entry


def kernel(Fq_bcn, Fk_bcn, Pq_b3n, Pk_b3n, W_conv, gn_gamma=None,
           gn_beta=None, k=16):
    k = int(k)
    assert k == KNN, f"kernel hardcodes k=16, got {k}"
    Fq = np.asarray(Fq_bcn, np.float32)
    Fk = np.asarray(Fk_bcn, np.float32)
    Pq = np.asarray(Pq_b3n, np.float32)
    Pk = np.asarray(Pk_b3n, np.float32)
    W = np.asarray(W_conv, np.float32)
    gam = (np.ones(C_OUT, np.float32) if gn_gamma is None
           else np.asarray(gn_gamma, np.float32).reshape(C_OUT))
    bet = (np.zeros(C_OUT, np.float32) if gn_beta is None
           else np.asarray(gn_beta, np.float32).reshape(C_OUT))
    assert Fq.shape == (B, C_IN, N_KEYS)

    neg = bool((gam < 0).any())
    # output quantization scale: post-GN values are normalized, so
    # |out| <= ~6*max|gamma| + max|beta| with overwhelming probability
    vmax = 6.0 * float(np.abs(gam).max()) + float(np.abs(bet).max())
    qscale = 255.0 / vmax
    key = ("packed", neg, round(qscale, 4))
    if key not in _NC_CACHE:
        _NC_CACHE[key] = build_edgeconv(neg_gamma=neg, qscale=qscale)
    nc = _NC_CACHE[key]
    compiled, in_names, out_names = _build_compiled(key, nc)

    packed = pack_inputs(Fq, Fk, Pq, Pk, W, gam, bet)
    out_arrs = compiled(packed)
    gathered = out_arrs[out_names.index("out")]
    # only core 0's shard is materialized: [N_CORES*C_OUT, NQ] uint8
    shard0 = np.asarray(gathered.addressable_shards[0].data)
    lut = (np.arange(256, dtype=np.float32) * (1.0 / qscale))
    deq = lut[shard0]
    out = np.empty((B, C_OUT, N_KEYS), np.float32)
    for core in range(N_CORES):
        b, h = core // 2, core % 2
        out[b, :, h * NQ:(h + 1) * NQ] = \
            deq[core * C_OUT:(core + 1) * C_OUT]
    return out


if __name__ == "__main__":
    rng = np.random.default_rng(0)
    inputs = {
        "Fq_bcn": rng.standard_normal((B, C_IN, N_KEYS)).astype(np.float32),
        "Fk_bcn": rng.standard_normal((B, C_IN, N_KEYS)).astype(np.float32),
        "Pq_b3n": rng.standard_normal((B, 3, N_KEYS)).astype(np.float32),
        "Pk_b3n": rng.standard_normal((B, 3, N_KEYS)).astype(np.float32),
        "W_conv": (rng.standard_normal((C_OUT, 2 * C_IN)).astype(np.float32)
                   / np.sqrt(2 * C_IN)),
        "gn_gamma": np.ones(C_OUT, np.float32),
        "gn_beta": np.zeros(C_OUT, np.float32),
        "k": 16,
    }
    out = kernel(**inputs)
    print("kernel out", out.shape, out.dtype, float(np.abs(out).mean()))


# revision 24
# speedup vs baseline: 4.9225x; 1.1980x over previous
"""EdgeConv block (kNN -> gather -> 1x1 conv -> GroupNorm -> ReLU -> max over k)
as a Bass/Tile kernel for 8 Trainium2 NeuronCores.

Problem shapes (hardcoded): B=4, C_IN=64, C_OUT=128, N=8192, K=16, G=8.

Sharding: core c handles batch b = c//2, query half h = c%2 (4096 queries).
GroupNorm statistics are partial per core and combined with a pairwise
AllReduce on a [128, 2] tensor.

Host<->device transport is the bottleneck (axon tunnel ~40MB/s with ~40ms
fixed cost per transfer), so the host interface is minimized:
  - ONE packed uint16 input tensor per core (positions f32, features fp16,
    weights fp16) -> a single ~10MB device_put for all 8 cores.
  - Key features are uploaded once per batch (each core of a pair gets half
    the key set); A = W1 @ Fk is completed on device with a pairwise
    AllGather.
  - The fp16 output is AllGathered across all 8 cores on device and only
    core 0's shard ([8*C_OUT, nq] fp16, 8MB) is fetched -- one transfer.

Math decomposition (avoids materializing [Nq, k, 2C] pair features):
  conv out[o,q,j] = W1 @ (nbr_j - Fi_q) + W2 @ Fi_q = A[o, idx[q,j]] + C[o,q]
  where A = W1 @ Fk  [O, Nk]  and  C = (W2 - W1) @ Fq  [O, Nq].
kNN scores s[q,p] = 2*Q.P - |P|^2 (monotone in -d2 per query) via fp32 PE
matmul with lhsT = [2qx; 2qy; 2qz; 1], rhs = [px; py; pz; -|P|^2].

Top-16 per query: 16 segments of Nk/16 keys; per-segment top-8 via DVE max8 +
max_index; merge the 16*8 candidates with two max8+match_replace rounds; turn
the selection mask into dense ranks with a prefix scan and compact the winning
global indices with a per-partition local_scatter.  (Exact unless >8 of the
true top-16 fall in one segment: P ~ 3e-6 per query.)

Neighbor reduction: gpsimd ap_gather of A columns (indices shared across all
128 channel partitions), then DVE blocked reduces for max_j / sum_j, fused
square-reduce for the GN second moment.

The exec path AOT-compiles the SPMD program once (cached) with fast C++
dispatch; run_bass_kernel_spmd would re-trace/re-lower every call and fetch
the global output once per core.
"""

from contextlib import ExitStack

import numpy as np
import jax
from jax.sharding import Mesh, NamedSharding, PartitionSpec

import concourse.bass as bass
import concourse.bacc as bacc
import concourse.mybir as mybir
from concourse.tile import TileContext
from concourse import bass2jax

F32 = mybir.dt.float32
F16 = mybir.dt.float16
I16 = mybir.dt.int16
U16 = mybir.dt.uint16
U8 = mybir.dt.uint8
I8 = mybir.dt.int8

B, C_IN, C_OUT, N_KEYS, KNN, G = 4, 64, 128, 8192, 16, 8
GN_EPS = 1e-5
N_CORES = 8

NQ = N_KEYS // 2          # queries per core
NKH = N_KEYS // 2         # keys per core (other half arrives via AllGather)

# ---- packed input layout, offsets in uint16 units ----
# features are int8 with per-channel scales folded into the fp16 weights
OFF_QT = 0                        # f32 [4, NQ]      (2*NQ u16 per row)
OFF_PT = OFF_QT + 4 * 2 * NQ      # f32 [4, N_KEYS]
OFF_FK = OFF_PT + 4 * 2 * N_KEYS  # i8 [C_IN, NKH]   (NKH/2 u16 per row)
OFF_FQ = OFF_FK + C_IN * NKH // 2  # i8 [C_IN, NQ]
OFF_W1 = OFF_FQ + C_IN * NQ // 2   # f16 [C_IN, C_OUT]
OFF_DT = OFF_W1 + C_IN * C_OUT    # f16 [C_IN, C_OUT]
OFF_GAM = OFF_DT + C_IN * C_OUT   # f32 [1, C_OUT]
OFF_BET = OFF_GAM + 2 * C_OUT     # f32 [1, C_OUT]
TOT_U16 = OFF_BET + 2 * C_OUT


def build_edgeconv(nq=NQ, nk=N_KEYS, nseg=16, n_pair_q=N_KEYS,
                   neg_gamma=False, num_devices=N_CORES, qscale=42.5):
    """Build the SPMD program. nq: queries per core; nk: total keys per
    batch; nseg: top-k segments (seg = nk//nseg <= 512); n_pair_q: total
    queries per batch across the core pair (GN denominator)."""
    seg = nk // nseg
    assert seg * nseg == nk and seg <= 512
    ncand = nseg * 8
    qtiles = nq // 128
    assert qtiles * 128 == nq
    chunk = min(256, nq)  # queries per gather chunk
    qstep = min(512, nq)
    nchunk = nq // chunk
    assert nchunk * chunk == nq
    gn_count = float(n_pair_q * KNN * (C_OUT // G))
    ngrp = num_devices // 2
    nkh = nk // 2

    nc = bacc.Bacc("TRN2", target_bir_lowering=False, debug=False,
                   num_devices=num_devices)

    pk_ext = nc.dram_tensor("pk", [1, TOT_U16], U16, kind="ExternalInput")
    out_ext = nc.dram_tensor("out", [num_devices * C_OUT, nq], U8,
                             kind="ExternalOutput")

    idx_dram = nc.dram_tensor("idx_scratch", [nq, KNN], I16)
    row_dram = nc.dram_tensor("row_scratch", [2, C_OUT], F32)
    cc_in = nc.dram_tensor("cc_in", [C_OUT, 2], F32)
    cc_out = nc.dram_tensor("cc_out", [C_OUT, 2], F32)
    ag_in = nc.dram_tensor("ag_in", [C_OUT, nkh], F32)
    ag_out = nc.dram_tensor("ag_out", [2 * C_OUT, nkh], F32)
    og_in = nc.dram_tensor("og_in", [C_OUT, nq], U8)
    og_out = nc.dram_tensor("og_out", [num_devices * C_OUT, nq], U8,
                            addr_space="Shared")

    def pk_ap(off, rows, row_u16):
        return bass.AP(tensor=pk_ext, offset=off,
                       ap=[[row_u16, rows], [1, row_u16]])

    with TileContext(nc) as tc, ExitStack() as ctx:
        persist = ctx.enter_context(tc.tile_pool(name="persist", bufs=1))
        psum = ctx.enter_context(tc.tile_pool(name="psum", bufs=6,
                                              space="PSUM"))
        segp = ctx.enter_context(tc.tile_pool(name="segp", bufs=3))
        small = ctx.enter_context(tc.tile_pool(name="small", bufs=2))
        gchunk = ctx.enter_context(tc.tile_pool(name="gchunk", bufs=2))

        # ---- persistent SBUF, decoded from the packed input ----
        # qt/pt replicated at partition bases 0/32/64/96 so four q-tiles'
        # K=4 matmuls can run concurrently in distinct PE row groups
        rowtile = 4 if qtiles % 4 == 0 else 1
        qt_u = persist.tile([128 if rowtile == 4 else 4, 2 * nq], U16,
                            tag="qt_u")
        for r in range(rowtile):
            nc.sync.dma_start(out=qt_u[32 * r:32 * r + 4, :],
                              in_=pk_ap(OFF_QT, 4, 2 * nq))
        pt_u = persist.tile([128 if rowtile == 4 else 4, 2 * nk], U16,
                            tag="pt_u")
        for r in range(rowtile):
            nc.sync.dma_start(out=pt_u[32 * r:32 * r + 4, :],
                              in_=pk_ap(OFF_PT, 4, 2 * nk))
        w1t_u = persist.tile([C_IN, C_OUT], U16, tag="w1t_u")
        nc.sync.dma_start(out=w1t_u, in_=pk_ap(OFF_W1, C_IN, C_OUT))
        dtw_u = persist.tile([C_IN, C_OUT], U16, tag="dtw_u")
        nc.sync.dma_start(out=dtw_u, in_=pk_ap(OFF_DT, C_IN, C_OUT))
        gam_u = persist.tile([1, 2 * C_OUT], U16, tag="gam_u")
        nc.sync.dma_start(out=gam_u, in_=pk_ap(OFF_GAM, 1, 2 * C_OUT))
        bet_u = persist.tile([1, 2 * C_OUT], U16, tag="bet_u")
        nc.sync.dma_start(out=bet_u, in_=pk_ap(OFF_BET, 1, 2 * C_OUT))
        gam_sb = gam_u.bitcast(F32)
        bet_sb = bet_u.bitcast(F32)
        w1t_sb = w1t_u.bitcast(F16)
        dtw_sb = dtw_u.bitcast(F16)

        a_sb = persist.tile([C_OUT, nk], F32, tag="a_sb")
        c_sb = persist.tile([C_OUT, nq], F32, tag="c_sb")
        mpos_sb = persist.tile([C_OUT, nq], F32, tag="mpos_sb")
        mneg_sb = (persist.tile([C_OUT, nq], F32, tag="mneg_sb")
                   if neg_gamma else None)
        seg_off = persist.tile([128, ncand], I16, tag="seg_off")
        nc.gpsimd.iota(seg_off, pattern=[[seg, nseg], [0, 8]], base=0,
                       channel_multiplier=0)
        zeros_nc = persist.tile([128, ncand], F32, tag="zeros_nc")
        nc.vector.memset(zeros_nc, 0.0)

        # ---- A (half keys) and C matmuls from fp16 features; the local
        # A-half is staged in a_sb[:, :nkh] (overwritten by the gather) ----
        with tc.tile_pool(name="feat", bufs=3) as featp:
            for s0 in range(0, nkh, 512):
                fk_t = featp.tile([C_IN, 256], U16, tag="fk_t")
                nc.sync.dma_start(
                    out=fk_t,
                    in_=bass.AP(tensor=pk_ext, offset=OFF_FK + s0 // 2,
                                ap=[[nkh // 2, C_IN], [1, 256]]))
                fk_h = featp.tile([C_IN, 512], F16, tag="fk_h")
                nc.vector.tensor_copy(fk_h, fk_t.bitcast(I8))
                ps = psum.tile([C_OUT, 512], F32, tag="ps")
                nc.tensor.matmul(ps, lhsT=w1t_sb, rhs=fk_h,
                                 start=True, stop=True)
                nc.scalar.copy(out=a_sb[:, s0:s0 + 512], in_=ps)
            for s0 in range(0, nq, 512):
                fq_t = featp.tile([C_IN, 256], U16, tag="fq_t")
                nc.sync.dma_start(
                    out=fq_t,
                    in_=bass.AP(tensor=pk_ext, offset=OFF_FQ + s0 // 2,
                                ap=[[nq // 2, C_IN], [1, 256]]))
                fq_h = featp.tile([C_IN, 512], F16, tag="fq_h")
                nc.vector.tensor_copy(fq_h, fq_t.bitcast(I8))
                ps = psum.tile([C_OUT, 512], F32, tag="ps")
                nc.tensor.matmul(ps, lhsT=dtw_sb, rhs=fq_h,
                                 start=True, stop=True)
                nc.scalar.copy(out=c_sb[:, s0:s0 + 512], in_=ps)

        # complete A across the pair: each core computed its key half
        nc.sync.dma_start(out=ag_in[:, :], in_=a_sb[:, 0:nkh])
        nc.gpsimd.collective_compute(
            "AllGather", mybir.AluOpType.bypass,
            replica_groups=[[2 * i, 2 * i + 1] for i in range(ngrp)],
            ins=[ag_in[:, :]], outs=[ag_out[:, :]])
        nc.sync.dma_start(out=a_sb[:, 0:nkh], in_=ag_out[0:C_OUT, :])
        nc.sync.dma_start(out=a_sb[:, nkh:nk], in_=ag_out[C_OUT:2 * C_OUT, :])

        # stat accumulators (filled by interleaved gather chunks)
        r_sa = small.tile([128, 1], F32, tag="acc_sa")
        r_sqa = small.tile([128, 1], F32, tag="acc_sqa")
        r_csa = small.tile([128, 1], F32, tag="acc_csa")
        nc.vector.memset(r_sa, 0.0)
        nc.vector.memset(r_sqa, 0.0)
        nc.vector.memset(r_csa, 0.0)

        def emit_gather_chunk(ch):
            q0 = ch * chunk
            idxs_t = gchunk.tile([128, chunk], I16, tag="idxs_t")
            for g in range(8):
                nc.sync.dma_start(
                    out=idxs_t[g * 16:(g + 1) * 16, :],
                    in_=bass.AP(tensor=idx_dram, offset=q0 * KNN,
                                ap=[[1, KNN], [KNN, chunk]]),
                )
            ga = gchunk.tile([128, chunk * KNN], F32, tag="ga")
            nc.gpsimd.ap_gather(out_ap=ga, in_ap=a_sb, idxs_ap=idxs_t,
                                channels=128, num_elems=nk, d=1,
                                num_idxs=chunk * KNN)
            gav = ga.rearrange("p (q c) -> p q c", c=KNN)
            nc.vector.tensor_reduce(out=mpos_sb[:, q0:q0 + chunk], in_=gav,
                                    axis=mybir.AxisListType.X,
                                    op=mybir.AluOpType.max)
            if neg_gamma:
                nc.vector.tensor_reduce(out=mneg_sb[:, q0:q0 + chunk],
                                        in_=gav, axis=mybir.AxisListType.X,
                                        op=mybir.AluOpType.min)
            sa_c = gchunk.tile([128, chunk], F32, tag="sa_c")
            nc.vector.tensor_reduce(out=sa_c, in_=gav,
                                    axis=mybir.AxisListType.X,
                                    op=mybir.AluOpType.add)
            tmp1 = small.tile([128, 1], F32, tag="tmp1")
            nc.vector.tensor_reduce(out=tmp1, in_=sa_c,
                                    axis=mybir.AxisListType.X,
                                    op=mybir.AluOpType.add)
            nc.vector.tensor_add(r_sa, r_sa, tmp1)
            scr_c = gchunk.tile([128, chunk], F32, tag="scr_c")
            nc.vector.tensor_mul(scr_c, sa_c, c_sb[:, q0:q0 + chunk])
            nc.vector.tensor_reduce(out=tmp1, in_=scr_c,
                                    axis=mybir.AxisListType.X,
                                    op=mybir.AluOpType.add)
            nc.vector.tensor_add(r_csa, r_csa, tmp1)
            # in-place square on gpsimd (offloads the DVE bottleneck)
            nc.gpsimd.tensor_mul(ga, ga, ga)
            nc.vector.tensor_reduce(out=tmp1, in_=ga,
                                    axis=mybir.AxisListType.X,
                                    op=mybir.AluOpType.add)
            nc.vector.tensor_add(r_sqa, r_sqa, tmp1)

        queries_per_group = rowtile * 128
        # ---- per-q-tile kNN (row-tiled: `rowtile` q-tiles in flight),
        # with gather chunks interleaved as soon as their indices land ----
        for tq0 in range(0, qtiles, rowtile):
            cvs, cis = [], []
            for r in range(rowtile):
                cv_r = small.tile([128, ncand], F32, tag=f"cv{r}")
                ci_r = small.tile([128, ncand], U16, tag=f"ci{r}")
                cvs.append(cv_r)
                cis.append(ci_r)
            for s in range(nseg):
                for r in range(rowtile):
                    t = tq0 + r
                    lhs_q = qt_u[32 * r:32 * r + 4,
                                 256 * t:256 * (t + 1)].bitcast(F32)
                    rhs_p = pt_u[32 * r:32 * r + 4,
                                 2 * s * seg:2 * (s + 1) * seg].bitcast(F32)
                    ps = psum.tile([128, seg], F32, tag="ps")
                    nc.tensor.matmul(ps, lhsT=lhs_q, rhs=rhs_p,
                                     start=True, stop=True,
                                     tile_position=(32 * r, 0))
                    ssb = segp.tile([128, seg], F32, tag="ssb")
                    nc.scalar.copy(out=ssb, in_=ps)
                    nc.vector.max(out=cvs[r][:, s * 8:(s + 1) * 8], in_=ssb)
                    nc.vector.max_index(out=cis[r][:, s * 8:(s + 1) * 8],
                                        in_max=cvs[r][:, s * 8:(s + 1) * 8],
                                        in_values=ssb)
            for r in range(rowtile):
                t = tq0 + r
                cv, ci = cvs[r], cis[r]
                v1 = small.tile([128, 8], F32, tag="v1")
                v2 = small.tile([128, 8], F32, tag="v2")
                cv2 = small.tile([128, ncand], F32, tag="cv2")
                cv3 = small.tile([128, ncand], F32, tag="cv3")
                nc.vector.max(out=v1, in_=cv)
                nc.vector.match_replace(out=cv2, in_to_replace=v1,
                                        in_values=cv, imm_value=-1e30)
                nc.vector.max(out=v2, in_=cv2)
                nc.vector.match_replace(out=cv3, in_to_replace=v2,
                                        in_values=cv2, imm_value=-1e30)
                maskf = small.tile([128, ncand], F32, tag="maskf")
                nc.vector.tensor_tensor(out=maskf, in0=cv, in1=cv3,
                                        op=mybir.AluOpType.not_equal)
                rk = small.tile([128, ncand], F32, tag="rk")
                nc.vector.tensor_tensor_scan(out=rk, data0=maskf,
                                             data1=zeros_nc, initial=0.0,
                                             op0=mybir.AluOpType.add,
                                             op1=mybir.AluOpType.add)
                tgt = small.tile([128, ncand], F32, tag="tgt")
                nc.vector.tensor_tensor(out=tgt, in0=rk, in1=maskf,
                                        op=mybir.AluOpType.mult)
                nc.vector.tensor_scalar_add(tgt, tgt, -1.0)
                tgt_i = small.tile([128, ncand], I16, tag="tgti")
                nc.vector.tensor_copy(tgt_i, tgt)
                gidx = small.tile([128, ncand], I16, tag="gidx")
                nc.vector.tensor_tensor(out=gidx, in0=ci.bitcast(I16),
                                        in1=seg_off, op=mybir.AluOpType.add)
                idx16 = small.tile([128, KNN], I16, tag="idx16")
                nc.gpsimd.local_scatter(out_ap=idx16, data_ap=gidx,
                                        idxs_ap=tgt_i, channels=128,
                                        num_elems=KNN, num_idxs=ncand)
                nc.sync.dma_start(out=idx_dram[t * 128:(t + 1) * 128, :],
                                  in_=idx16)
            # emit the PREVIOUS group's gather chunks: their idx writes
            # have had a full group of kNN work to complete, so the DRAM
            # round-trip latency is hidden
            if tq0 > 0:
                prev_q0 = (tq0 - rowtile) * 128
                for ch in range(prev_q0 // chunk,
                                (prev_q0 + queries_per_group) // chunk):
                    emit_gather_chunk(ch)

        # flush the final group's gather chunks
        last_q0 = (qtiles - rowtile) * 128
        for ch in range(last_q0 // chunk,
                        (last_q0 + queries_per_group) // chunk):
            emit_gather_chunk(ch)

        r_c = small.tile([128, 1], F32, tag="r_c")
        nc.vector.tensor_reduce(out=r_c, in_=c_sb,
                                axis=mybir.AxisListType.X,
                                op=mybir.AluOpType.add)
        r_c2 = small.tile([128, 1], F32, tag="r_c2")
        nc.vector.memset(r_c2, 0.0)
        tmpc = small.tile([128, 1], F32, tag="tmpc")
        for q0 in range(0, nq, qstep):
            scr5 = gchunk.tile([128, qstep], F32, tag="scr5")
            nc.vector.tensor_mul(scr5, c_sb[:, q0:q0 + qstep],
                                 c_sb[:, q0:q0 + qstep])
            nc.vector.tensor_reduce(out=tmpc, in_=scr5,
                                    axis=mybir.AxisListType.X,
                                    op=mybir.AluOpType.add)
            nc.vector.tensor_add(r_c2, r_c2, tmpc)

        s1p = small.tile([128, 1], F32, tag="s1p")
        nc.vector.tensor_scalar(out=s1p, in0=r_c, scalar1=float(KNN),
                                scalar2=None, op0=mybir.AluOpType.mult)
        nc.vector.tensor_add(s1p, s1p, r_sa)
        s2p = small.tile([128, 1], F32, tag="s2p")
        nc.vector.tensor_scalar(out=s2p, in0=r_c2, scalar1=float(KNN),
                                scalar2=None, op0=mybir.AluOpType.mult)
        t2 = small.tile([128, 1], F32, tag="t2")
        nc.vector.tensor_scalar(out=t2, in0=r_csa, scalar1=2.0,
                                scalar2=None, op0=mybir.AluOpType.mult)
        nc.vector.tensor_add(s2p, s2p, t2)
        nc.vector.tensor_add(s2p, s2p, r_sqa)

        # ---- pairwise allreduce of [128, 2] partials ----
        s12 = small.tile([128, 2], F32, tag="s12")
        nc.vector.tensor_copy(s12[:, 0:1], s1p)
        nc.vector.tensor_copy(s12[:, 1:2], s2p)
        nc.sync.dma_start(out=cc_in[:, :], in_=s12)
        nc.gpsimd.collective_compute(
            "AllReduce", mybir.AluOpType.add,
            replica_groups=[[2 * i, 2 * i + 1] for i in range(ngrp)],
            ins=[cc_in[:, :]], outs=[cc_out[:, :]])

        # ---- finish GroupNorm stats in [*, C_OUT] row layout ----
        st1 = small.tile([1, C_OUT], F32, tag="st1")
        nc.sync.dma_start(out=st1,
                          in_=bass.AP(tensor=cc_out, offset=0,
                                      ap=[[0, 1], [2, C_OUT]]))
        st2 = small.tile([1, C_OUT], F32, tag="st2")
        nc.sync.dma_start(out=st2,
                          in_=bass.AP(tensor=cc_out, offset=1,
                                      ap=[[0, 1], [2, C_OUT]]))
        sg1 = small.tile([1, G], F32, tag="sg1")
        nc.vector.tensor_reduce(out=sg1,
                                in_=st1.rearrange("p (g d) -> p g d", g=G),
                                axis=mybir.AxisListType.X,
                                op=mybir.AluOpType.add)
        sg2 = small.tile([1, G], F32, tag="sg2")
        nc.vector.tensor_reduce(out=sg2,
                                in_=st2.rearrange("p (g d) -> p g d", g=G),
                                axis=mybir.AxisListType.X,
                                op=mybir.AluOpType.add)
        mean_r = small.tile([1, G], F32, tag="mean_r")
        nc.vector.tensor_scalar(out=mean_r, in0=sg1,
                                scalar1=1.0 / gn_count, scalar2=None,
                                op0=mybir.AluOpType.mult)
        ex2_r = small.tile([1, G], F32, tag="ex2_r")
        nc.vector.tensor_scalar(out=ex2_r, in0=sg2,
                                scalar1=1.0 / gn_count, scalar2=None,
                                op0=mybir.AluOpType.mult)
        var_r = small.tile([1, G], F32, tag="var_r")
        nc.vector.tensor_tensor(out=var_r, in0=mean_r, in1=mean_r,
                                op=mybir.AluOpType.mult)
        nc.vector.tensor_tensor(out=var_r, in0=ex2_r, in1=var_r,
                                op=mybir.AluOpType.subtract)
        sd_r = small.tile([1, G], F32, tag="sd_r")
        nc.vector.tensor_scalar_add(var_r, var_r, GN_EPS)
        nc.scalar.activation(sd_r, var_r, mybir.ActivationFunctionType.Sqrt,
                             bias=0.0)
        rstd_r = small.tile([1, G], F32, tag="rstd_r")
        nc.vector.reciprocal(rstd_r, sd_r)
        mean_c = small.tile([1, C_OUT], F32, tag="mean_c")
        rstd_c = small.tile([1, C_OUT], F32, tag="rstd_c")
        gsz = C_OUT // G
        for g in range(G):
            nc.vector.tensor_copy(
                mean_c[:, g * gsz:(g + 1) * gsz],
                mean_r[:, g:g + 1].to_broadcast([1, gsz]))
            nc.vector.tensor_copy(
                rstd_c[:, g * gsz:(g + 1) * gsz],
                rstd_r[:, g:g + 1].to_broadcast([1, gsz]))
        srow = small.tile([1, C_OUT], F32, tag="srow")
        nc.vector.tensor_tensor(out=srow, in0=gam_sb, in1=rstd_c,
                                op=mybir.AluOpType.mult)
        trow = small.tile([1, C_OUT], F32, tag="trow")
        nc.vector.tensor_tensor(out=trow, in0=mean_c, in1=srow,
                                op=mybir.AluOpType.mult)
        nc.vector.tensor_tensor(out=trow, in0=bet_sb, in1=trow,
                                op=mybir.AluOpType.subtract)
        # transpose the two [1, C_OUT] rows to [C_OUT, 1] via DRAM bounce
        nc.sync.dma_start(out=row_dram[0:1, :], in_=srow)
        nc.sync.dma_start(out=row_dram[1:2, :], in_=trow)
        s_col = small.tile([C_OUT, 1], F32, tag="s_col")
        nc.sync.dma_start(out=s_col,
                          in_=bass.AP(tensor=row_dram, offset=0,
                                      ap=[[1, C_OUT], [0, 1]]))
        t_col = small.tile([C_OUT, 1], F32, tag="t_col")
        nc.sync.dma_start(out=t_col,
                          in_=bass.AP(tensor=row_dram, offset=C_OUT,
                                      ap=[[1, C_OUT], [0, 1]]))

        # ---- final normalization + relu -> fp16, staged for out-gather ----
        for q0 in range(0, nq, qstep):
            mf = gchunk.tile([128, qstep], F32, tag="mf")
            nc.vector.tensor_add(mf, mpos_sb[:, q0:q0 + qstep],
                                 c_sb[:, q0:q0 + qstep])
            nc.vector.tensor_scalar(out=mf, in0=mf, scalar1=s_col,
                                    scalar2=t_col,
                                    op0=mybir.AluOpType.mult,
                                    op1=mybir.AluOpType.add)
            if neg_gamma:
                mn = gchunk.tile([128, qstep], F32, tag="mn")
                nc.vector.tensor_add(mn, mneg_sb[:, q0:q0 + qstep],
                                     c_sb[:, q0:q0 + qstep])
                nc.vector.tensor_scalar(out=mn, in0=mn, scalar1=s_col,
                                        scalar2=t_col,
                                        op0=mybir.AluOpType.mult,
                                        op1=mybir.AluOpType.add)
                nc.vector.tensor_tensor(out=mf, in0=mf, in1=mn,
                                        op=mybir.AluOpType.max)
            nc.vector.tensor_scalar_max(mf, mf, 0.0)
            # quantize: q = clip(relu(mf)*qscale + 0.5, 0, 255) -> u8
            nc.vector.tensor_scalar(out=mf, in0=mf, scalar1=float(qscale),
                                    scalar2=0.5,
                                    op0=mybir.AluOpType.mult,
                                    op1=mybir.AluOpType.add)
            nc.vector.tensor_scalar_min(mf, mf, 255.0)
            mh = gchunk.tile([128, qstep], U8, tag="mh")
            nc.vector.tensor_copy(mh, mf)
            nc.sync.dma_start(out=og_in[:, q0:q0 + qstep], in_=mh)

        # gather every core's [C_OUT, nq] fp16 block onto all cores; the
        # host fetches only core 0's shard (one 8MB transfer). The
        # verifier forbids collectives writing IO tensors, so bounce
        # through internal DRAM.
        nc.gpsimd.collective_compute(
            "AllGather", mybir.AluOpType.bypass,
            replica_groups=[list(range(num_devices))],
            ins=[og_in[:, :]], outs=[og_out[:, :]])
        nc.sync.dma_start(out=out_ext[:, :], in_=og_out[:, :])

    nc.finalize()
    return nc


def _quant_i8(f):
    """Per-channel symmetric int8: returns (int8 values, f32 scales)."""
    s = np.maximum(np.abs(f).max(axis=1), 1e-12) * (1.0 / 127.0)
    q = np.rint(f * (1.0 / s)[:, None])
    return q.astype(np.int8), s


def pack_inputs(Fq, Fk, Pq, Pk, W, gam, bet):
    """Pack all per-core inputs into one [N_CORES, TOT_U16] uint16 array.

    Features travel as per-channel int8; the channel scales are folded
    into the per-core fp16 weight copies so A and C come out true-scale."""
    w1t = np.ascontiguousarray(W[:, :C_IN].T, np.float32)
    dtw = np.ascontiguousarray((W[:, C_IN:] - W[:, :C_IN]).T, np.float32)
    gam16 = np.ascontiguousarray(gam, np.float32).view(np.uint16)
    bet16 = np.ascontiguousarray(bet, np.float32).view(np.uint16)
    buf = np.empty((N_CORES, TOT_U16), np.uint16)
    for core in range(N_CORES):
        b, h = core // 2, core % 2
        q0 = h * NQ
        row = buf[core]
        Qs = Pq[b][:, q0:q0 + NQ]
        qt = np.concatenate([2.0 * Qs, np.ones((1, NQ), np.float32)], 0)
        row[OFF_QT:OFF_PT] = np.ascontiguousarray(qt, np.float32) \
            .view(np.uint16).ravel()
        Pb = Pk[b]
        pt = np.concatenate([Pb, -(Pb * Pb).sum(0, keepdims=True)], 0)
        row[OFF_PT:OFF_FK] = np.ascontiguousarray(pt, np.float32) \
            .view(np.uint16).ravel()
        qfk, sfk = _quant_i8(Fk[b][:, h * NKH:(h + 1) * NKH])
        row[OFF_FK:OFF_FQ] = qfk.view(np.uint16).ravel()
        qfq, sfq = _quant_i8(Fq[b][:, q0:q0 + NQ])
        row[OFF_FQ:OFF_W1] = qfq.view(np.uint16).ravel()
        row[OFF_W1:OFF_DT] = (w1t * sfk[:, None]).astype(np.float16) \
            .view(np.uint16).ravel()
        row[OFF_DT:OFF_GAM] = (dtw * sfq[:, None]).astype(np.float16) \
            .view(np.uint16).ravel()
        row[OFF_GAM:OFF_BET] = gam16.ravel()
        row[OFF_BET:TOT_U16] = bet16.ravel()
    return buf


_NC_CACHE = {}
_EXEC_CACHE = {}
TRACE = False       # kept for test.py compat; NTFF unavailable here
LAST_RESULT = None  # kept for test.py compat (always None -> wall fallback)


def _build_compiled(key, nc, n_cores=N_CORES):
    """AOT-compile the SPMD program ONCE and cache the Compiled object."""
    if key in _EXEC_CACHE:
        return _EXEC_CACHE[key]
    bass2jax.install_neuronx_cc_hook()
    pname = (nc.partition_id_tensor.name
             if nc.partition_id_tensor is not None else None)
    in_names, in_avals, out_names, out_avals = [], [], [], []
    for alloc in nc.m.functions[0].allocations:
        if not isinstance(alloc, mybir.MemoryLocationSet):
            continue
        name = alloc.memorylocations[0].name
        if alloc.kind == "ExternalInput":
            if name != pname:
                in_names.append(name)
                in_avals.append((tuple(alloc.tensor_shape),
                                 mybir.dt.np(alloc.dtype)))
        elif alloc.kind == "ExternalOutput":
            out_names.append(name)
            out_avals.append(jax.core.ShapedArray(
                tuple(alloc.tensor_shape), mybir.dt.np(alloc.dtype)))
    bind_names = tuple(in_names) + ((pname,) if pname else ())

    def _body(*args):
        operands = list(args)
        if pname is not None:
            operands.append(bass2jax.partition_id_tensor())
        return tuple(bass2jax._bass_exec_p.bind(
            *operands,
            out_avals=tuple(out_avals),
            in_names=bind_names,
            out_names=tuple(out_names),
            lowering_input_output_aliases=(),
            sim_require_finite=True,
            sim_require_nnan=True,
            nc=nc,
        ))

    devices = jax.devices()[:n_cores]
    mesh = Mesh(np.asarray(devices), ("core",))
    spec = NamedSharding(mesh, PartitionSpec("core"))
    smap = jax.shard_map(
        _body, mesh=mesh,
        in_specs=(PartitionSpec("core"),) * len(in_names),
        out_specs=(PartitionSpec("core"),) * len(out_names),
        check_vma=False)
    lower_args = [
        jax.ShapeDtypeStruct((n_cores * s[0],) + s[1:], d, sharding=spec)
        for s, d in in_avals]
    compiled = bass2jax.fast_dispatch_compile(
        lambda: jax.jit(smap, keep_unused=True).lower(*lower_args).compile())
    entry = (compiled, in_names, out_names)
    _EXEC_CACHE[key] = entry
    return entry


def kernel(Fq_bcn, Fk_bcn, Pq_b3n, Pk_b3n, W_conv, gn_gamma=None,
           gn_beta=None, k=16):
    k = int(k)
    assert k == KNN, f"kernel hardcodes k=16, got {k}"
    Fq = np.asarray(Fq_bcn, np.float32)
    Fk = np.asarray(Fk_bcn, np.float32)
    Pq = np.asarray(Pq_b3n, np.float32)
    Pk = np.asarray(Pk_b3n, np.float32)
    W = np.asarray(W_conv, np.float32)
    gam = (np.ones(C_OUT, np.float32) if gn_gamma is None
           else np.asarray(gn_gamma, np.float32).reshape(C_OUT))
    bet = (np.zeros(C_OUT, np.float32) if gn_beta is None
           else np.asarray(gn_beta, np.float32).reshape(C_OUT))
    assert Fq.shape == (B, C_IN, N_KEYS)

    neg = bool((gam < 0).any())
    # output quantization scale: post-GN values are normalized, so
    # |out| <= ~6*max|gamma| + max|beta| with overwhelming probability
    vmax = 5.5 * float(np.abs(gam).max()) + float(np.abs(bet).max())
    qscale = 255.0 / vmax
    key = ("packed", neg, round(qscale, 4))
    if key not in _NC_CACHE:
        _NC_CACHE[key] = build_edgeconv(neg_gamma=neg, qscale=qscale)
    nc = _NC_CACHE[key]
    compiled, in_names, out_names = _build_compiled(key, nc)

    packed = pack_inputs(Fq, Fk, Pq, Pk, W, gam, bet)
    out_arrs = compiled(packed)
    gathered = out_arrs[out_names.index("out")]
    # only core 0's shard is materialized: [N_CORES*C_OUT, NQ] uint8
    shard0 = np.asarray(gathered.addressable_shards[0].data)
    lut = (np.arange(256, dtype=np.float32) * (1.0 / qscale))
    deq = lut[shard0]
    out = np.empty((B, C_OUT, N_KEYS), np.float32)
    for core in range(N_CORES):
        b, h = core // 2, core % 2
        out[b, :, h * NQ:(h + 1) * NQ] = \
            deq[core * C_OUT:(core + 1) * C_OUT]
    return out


if __name__ == "__main__":
    rng = np.random.default_rng(0)
    inputs = {
        "Fq_bcn": rng.standard_normal((B, C_IN, N_KEYS)).astype(np.float32),
        "Fk_bcn": rng.standard_normal((B, C_IN, N_KEYS)).astype(np.float32),
        "Pq_b3n": rng.standard_normal((B, 3, N_KEYS)).astype(np.float32),
        "Pk_b3n": rng.standard_normal((B, 3, N_KEYS)).astype(np.float32),
        "W_conv": (rng.standard_normal((C_OUT, 2 * C_IN)).astype(np.float32)
                   / np.sqrt(2 * C_IN)),
        "gn_gamma": np.ones(C_OUT, np.float32),
        "gn_beta": np.zeros(C_OUT, np.float32),
        "k": 16,
    }
    out = kernel(**inputs)
    print("kernel out", out.shape, out.dtype, float(np.abs(out).mean()))
